# revision 1
# baseline (speedup 1.0000x reference)
"""Trainium2 Bass kernel for a GQA attention block (LuluAttention).

Problem: hidden_states [2, 2048, 2048], 16 q heads / 4 kv heads of dim 128,
RoPE, softmax attention, output projection.

Sharding: 8 cores = 2 (batch) x 4 (query-row blocks of 512 rows).
Each core computes the full K/V for its batch (all 4 kv heads), Q for its
512-row query slice (all 16 heads), RoPE, attention, and the output
projection for its row slice. The full output is assembled on the host by
pure concatenation (no collectives needed).

Device-side layout: everything is kept transposed ([head_dim, seq] with
head_dim on SBUF partitions):
  - QT/KT come straight out of matmul(lhsT=W_slice, rhs=hsT)
  - scores are computed transposed: scoresT = K @ Q^T
  - exp(scoresT) feeds the AV matmul directly (lhsT = V tile natural)
  - softmax denominator = ones128 @ expT (broadcast across partitions)
  - ctxT slices are directly the lhsT for the output projection
so no on-device transposes are needed anywhere.  hs^T is prepared on the
host as part of input sharding.
"""

import os
import sys

if "/opt/trn_rl_repo" not in sys.path:
    sys.path.insert(0, "/opt/trn_rl_repo")

import numpy as np

B, S, H = 2, 2048, 2048
NH, NKV, D = 16, 4, 128
SQ = 512          # query rows per core
NCORES = 8
P = 128
NT = H // P       # 16 contraction tiles over hidden dim
ROPE_THETA = 10000.0
SCALE = 1.0 / float(np.sqrt(D))


def _rope_tables_T():
    """cosT/ssinT [D, S]: transposed RoPE tables with the rotate-half sign
    folded into ssin (negative for d<64)."""
    inv_freq = 1.0 / (ROPE_THETA ** (np.arange(0, D, 2, dtype=np.float64) / D))
    t = np.arange(S, dtype=np.float64)
    freqs = np.outer(t, inv_freq)                     # [S, D/2]
    emb = np.concatenate([freqs, freqs], axis=-1)     # [S, D]
    cos = np.cos(emb).astype(np.float32)
    sin = np.sin(emb).astype(np.float32)
    ssin = sin.copy()
    ssin[:, : D // 2] *= -1.0
    return np.ascontiguousarray(cos.T), np.ascontiguousarray(ssin.T)


def _build_program():
    from concourse import bacc, mybir, tile

    F32 = mybir.dt.float32
    F32R = mybir.dt.float32r
    AF = mybir.ActivationFunctionType

    def r(ap):
        # Plain fp32 matmul: the BIR verifier in this deployment rejects
        # f32->f32r bitcasts of DMA-written tiles ("not rounded to FP32r").
        return ap

    nc = bacc.Bacc(
        "TRN2", target_bir_lowering=False, debug=False, num_devices=NCORES
    )

    # f32r end-to-end for the projection operands: DMA'd f32r tiles are
    # legal FP32r matmul inputs (4x PE rate vs fp32), and f32r's numpy
    # binding is still float32 so the host side is unchanged.
    hsT = nc.dram_tensor("hsT", [H, S], F32R, kind="ExternalInput").ap()
    hsQ = nc.dram_tensor("hsQ", [H, SQ], F32R, kind="ExternalInput").ap()
    wq = nc.dram_tensor("wq", [H, NH * D], F32R, kind="ExternalInput").ap()
    wk = nc.dram_tensor("wk", [H, NKV * D], F32R, kind="ExternalInput").ap()
    wv = nc.dram_tensor("wv", [H, NKV * D], F32R, kind="ExternalInput").ap()
    wo = nc.dram_tensor("wo", [NH * D, H], F32, kind="ExternalInput").ap()
    bqT = nc.dram_tensor("bqT", [D, NH], F32, kind="ExternalInput").ap()
    bkT = nc.dram_tensor("bkT", [D, NKV], F32, kind="ExternalInput").ap()
    bv = nc.dram_tensor("bv", [1, NKV * D], F32, kind="ExternalInput").ap()
    cosq = nc.dram_tensor("cosq", [D, SQ], F32, kind="ExternalInput").ap()
    ssinq = nc.dram_tensor("ssinq", [D, SQ], F32, kind="ExternalInput").ap()
    cosk = nc.dram_tensor("cosk", [D, S], F32, kind="ExternalInput").ap()
    ssink = nc.dram_tensor("ssink", [D, S], F32, kind="ExternalInput").ap()
    out = nc.dram_tensor("out", [SQ, H], F32, kind="ExternalOutput").ap()

    with tile.TileContext(nc) as tc:
        # ---- long-lived pools (explicit alloc/release for phase-scoped
        # lifetimes that don't nest cleanly) ----
        # left side: long-lived (cst, kvp, qp); right side: phase scratch.
        # Each side is a stack — pools must be released in LIFO order.
        cst = tc.alloc_tile_pool(name="cst", bufs=1)
        kvp = tc.alloc_tile_pool(name="kvp", bufs=1)     # kt + vt (K..phase2)
        rp = tc.alloc_tile_pool(name="rp", bufs=2, side="right")  # rope scratch

        ones1 = cst.tile([1, P], F32, tag="ones1")
        nc.gpsimd.memset(ones1[:], 1.0)
        ones128 = cst.tile([P, P], F32, tag="ones128")
        nc.gpsimd.memset(ones128[:], 1.0)
        bqT_sb = cst.tile([D, NH], F32, tag="bqT")
        nc.sync.dma_start(bqT_sb[:], bqT[:, :])
        bkT_sb = cst.tile([D, NKV], F32, tag="bkT")
        nc.sync.dma_start(bkT_sb[:], bkT[:, :])
        bv_sb = cst.tile([1, NKV * D], F32, tag="bv")
        nc.sync.dma_start(bv_sb[:], bv[:, :])

        kt = [res_t for res_t in (
            kvp.tile([D, S], F32, tag=f"kt{g}", name=f"kt{g}")
            for g in range(NKV)
        )]
        vt = [res_t for res_t in (
            kvp.tile([P, NKV * D], F32, tag=f"v{t}", name=f"v{t}")
            for t in range(S // P)
        )]

        def rope(dst, tbl_cos, tbl_sin, width):
            """In-place RoPE on dst [128, width] (transposed layout)."""
            sh = rp.tile([P, 512], F32, tag="sh", name="sh")
            nc.sync.dma_start(sh[0 : D // 2, :width], dst[D // 2 : D, :])
            nc.sync.dma_start(sh[D // 2 : D, :width], dst[0 : D // 2, :])
            t1 = rp.tile([P, 512], F32, tag="rt1", name="rt1")
            nc.vector.tensor_mul(t1[:, :width], sh[:, :width], tbl_sin)
            t2 = rp.tile([P, 512], F32, tag="rt2", name="rt2")
            nc.vector.tensor_mul(t2[:, :width], dst[:], tbl_cos)
            nc.vector.tensor_add(dst[:], t1[:, :width], t2[:, :width])

        # ================= stage K =================
        # KT[g] [d=128, s2=2048] = (hs @ Wk + bk)^T, rope'd.
        # Two sweeps over hsT (g pairs) with 8 PSUM banks live each.
        pk = tc.alloc_tile_pool(name="ps_k", bufs=8, space="PSUM")
        wsk = tc.alloc_tile_pool(name="wsk", bufs=1, side="right")
        for sweep in range(2):
            gs = (2 * sweep, 2 * sweep + 1)
            banks = {
                (g, c): pk.tile([P, 512], F32, tag="pj", name=f"pk_{g}_{c}")
                for g in gs
                for c in range(4)
            }
            for ht in range(NT):
                hst = wsk.tile([P, S], F32R, tag="hs", bufs=3, name="hst")
                nc.sync.dma_start(hst[:], hsT[ht * P : (ht + 1) * P, :])
                for g in gs:
                    wkt = wsk.tile([P, P], F32R, tag="wk", bufs=4, name="wkt")
                    nc.sync.dma_start(
                        wkt[:],
                        wk[ht * P : (ht + 1) * P, g * D : (g + 1) * D],
                    )
                    for c in range(4):
                        nc.tensor.matmul(
                            banks[(g, c)][:],
                            r(wkt[:]),
                            r(hst[:, c * 512 : (c + 1) * 512]),
                            start=(ht == 0),
                            stop=(ht == NT - 1),
                        )
            for g in gs:
                for c in range(4):
                    # copy + bias (bk varies along partitions here)
                    nc.scalar.activation(
                        kt[g][:, c * 512 : (c + 1) * 512],
                        banks[(g, c)][:],
                        AF.Identity,
                        bias=bkT_sb[:, g : g + 1],
                    )
            # rope per 512-chunk; share table tiles across g
            for c in range(4):
                ck = wsk.tile([P, 512], F32, tag="tbc", bufs=2, name="ck")
                nc.sync.dma_start(ck[:], cosk[:, c * 512 : (c + 1) * 512])
                sk = wsk.tile([P, 512], F32, tag="tbs", bufs=2, name="sk")
                nc.sync.dma_start(sk[:], ssink[:, c * 512 : (c + 1) * 512])
                for g in gs:
                    rope(kt[g][:, c * 512 : (c + 1) * 512], ck[:], sk[:], 512)
        wsk.release()

        # ================= stage V =================
        # V[t] [s2-tile=128, 4*128] = hs @ Wv + bv (natural layout).
        wvp = tc.alloc_tile_pool(name="wvp", bufs=1, side="right")
        wsv = tc.alloc_tile_pool(name="wsv", bufs=4, side="right")
        wvres = [
            wvp.tile([P, NKV * D], F32R, tag=f"wv{ht}", name=f"wv{ht}")
            for ht in range(NT)
        ]
        for ht in range(NT):
            nc.sync.dma_start(wvres[ht][:], wv[ht * P : (ht + 1) * P, :])
        for t in range(S // P):
            bank = pk.tile([P, 512], F32, tag="pj", name=f"pv_{t}")
            for ht in range(NT):
                hsl = wsv.tile([P, P], F32R, tag="hsv", name="hsl")
                nc.sync.dma_start(
                    hsl[:],
                    hsT[ht * P : (ht + 1) * P, t * P : (t + 1) * P],
                )
                nc.tensor.matmul(
                    bank[:],
                    r(hsl[:]),
                    r(wvres[ht][:]),
                    start=(ht == 0),
                    stop=False,
                )
            nc.tensor.matmul(
                bank[:], r(ones1[:]), r(bv_sb[:]), start=False, stop=True
            )
            nc.scalar.copy(vt[t][:], bank[:])
        wsv.release()
        wvp.release()

        # ================= stage Q =================
        # QT[h] [d=128, s1=512] = (hs_q @ Wq + bq)^T, rope'd.
        qp = tc.alloc_tile_pool(name="qp", bufs=1)       # qt (Q..phase2)
        hsqp = tc.alloc_tile_pool(name="hsqp", bufs=1, side="right")
        wsq = tc.alloc_tile_pool(name="wsq", bufs=4, side="right")
        qt = [
            qp.tile([D, SQ], F32, tag=f"qt{h}", name=f"qt{h}")
            for h in range(NH)
        ]
        cosq_sb = hsqp.tile([D, SQ], F32, tag="cosq")
        nc.sync.dma_start(cosq_sb[:], cosq[:, :])
        ssinq_sb = hsqp.tile([D, SQ], F32, tag="ssinq")
        nc.sync.dma_start(ssinq_sb[:], ssinq[:, :])
        hsqres = [
            hsqp.tile([P, SQ], F32R, tag=f"hsq{ht}", name=f"hsq{ht}")
            for ht in range(NT)
        ]
        for ht in range(NT):
            nc.sync.dma_start(hsqres[ht][:], hsQ[ht * P : (ht + 1) * P, :])
        for h in range(NH):
            bank = pk.tile([P, 512], F32, tag="pj", name=f"pq_{h}")
            for ht in range(NT):
                wqt = wsq.tile([P, P], F32R, tag="wq", name="wqt")
                nc.sync.dma_start(
                    wqt[:],
                    wq[ht * P : (ht + 1) * P, h * D : (h + 1) * D],
                )
                nc.tensor.matmul(
                    bank[:],
                    r(wqt[:]),
                    r(hsqres[ht][:]),
                    start=(ht == 0),
                    stop=(ht == NT - 1),
                )
            nc.scalar.activation(
                qt[h][:], bank[:], AF.Identity, bias=bqT_sb[:, h : h + 1]
            )
            rope(qt[h], cosq_sb[:], ssinq_sb[:], SQ)
        wsq.release()
        hsqp.release()
        rp.release()
        pk.release()

        # ================= phase 2: attention =================
        pa = tc.alloc_tile_pool(name="ps_a", bufs=5, space="PSUM")
        pc = tc.alloc_tile_pool(name="ps_c", bufs=3, space="PSUM")
        ctxp = tc.alloc_tile_pool(name="ctxp", bufs=1, side="right")
        wsa = tc.alloc_tile_pool(name="wsa", bufs=4, side="right")
        ctx = [
            ctxp.tile([D, SQ], F32, tag=f"ctx{h}", name=f"ctx{h}")
            for h in range(NH)
        ]
        for h in range(NH):
            g = h // (NH // NKV)
            ctx_ps = pc.tile([P, SQ], F32, tag="acc", name=f"ctxps{h}")
            den_ps = pc.tile([P, SQ], F32, tag="acc", name=f"denps{h}")
            for t in range(S // P):
                sc = pa.tile([P, SQ], F32, tag="sc", bufs=5, name="sc")
                nc.tensor.matmul(
                    sc[:],
                    r(kt[g][:, t * P : (t + 1) * P]),
                    r(qt[h][:]),
                    start=True,
                    stop=True,
                )
                at = wsa.tile([P, SQ], F32, tag="at", name="at")
                nc.scalar.activation(at[:], sc[:], AF.Exp, scale=SCALE)
                nc.tensor.matmul(
                    ctx_ps[:],
                    r(vt[t][:, g * D : (g + 1) * D]),
                    r(at[:]),
                    start=(t == 0),
                    stop=(t == S // P - 1),
                )
                nc.tensor.matmul(
                    den_ps[:],
                    r(ones128[:]),
                    r(at[:]),
                    start=(t == 0),
                    stop=(t == S // P - 1),
                )
            rc = wsa.tile([P, SQ], F32, tag="rc", bufs=2, name="rc")
            nc.vector.reciprocal(rc[:], den_ps[:])
            nc.vector.tensor_mul(ctx[h][:], ctx_ps[:], rc[:])
        wsa.release()
        qp.release()
        kvp.release()
        pc.release()
        pa.release()

        # ================= phase 3: output projection =================
        po = tc.alloc_tile_pool(name="ps_o", bufs=8, space="PSUM")
        wso = tc.alloc_tile_pool(name="wso", bufs=3, side="right")
        for hc in range(4):
            banks = [
                po.tile([P, 512], F32, tag="po", name=f"po_{hc}_{i}")
                for i in range(4)
            ]
            for h in range(NH):
                wot = wso.tile([P, 512], F32, tag="wo", name="wot")
                nc.sync.dma_start(
                    wot[:],
                    wo[h * D : (h + 1) * D, hc * 512 : (hc + 1) * 512],
                )
                for s1t in range(4):
                    nc.tensor.matmul(
                        banks[s1t][:],
                        r(ctx[h][:, s1t * P : (s1t + 1) * P]),
                        r(wot[:]),
                        start=(h == 0),
                        stop=(h == NH - 1),
                    )
            for s1t in range(4):
                ob = wso.tile([P, 512], F32, tag="ob", name="ob")
                nc.scalar.copy(ob[:], banks[s1t][:])
                nc.sync.dma_start(
                    out[s1t * P : (s1t + 1) * P, hc * 512 : (hc + 1) * 512],
                    ob[:],
                )
        wso.release()
        po.release()
        ctxp.release()
        cst.release()

    nc.compile()
    return nc


_PROGRAM_CACHE = {}


def _get_program():
    if "nc" not in _PROGRAM_CACHE:
        _PROGRAM_CACHE["nc"] = _build_program()
    return _PROGRAM_CACHE["nc"]


def _prepare_in_maps(hidden_states, Wq, bq, Wk, bk, Wv, bv, Wo):
    hidden_states = np.asarray(hidden_states, dtype=np.float32)
    Wq = np.asarray(Wq, dtype=np.float32)
    bq = np.asarray(bq, dtype=np.float32)
    Wk = np.asarray(Wk, dtype=np.float32)
    bk = np.asarray(bk, dtype=np.float32)
    Wv = np.asarray(Wv, dtype=np.float32)
    bv = np.asarray(bv, dtype=np.float32)
    Wo = np.asarray(Wo, dtype=np.float32)

    cosT, ssinT = _rope_tables_T()
    bqT_h = np.ascontiguousarray(bq.reshape(NH, D).T)    # [128, 16]
    bkT_h = np.ascontiguousarray(bk.reshape(NKV, D).T)   # [128, 4]
    bv_h = bv.reshape(1, NKV * D)

    hsT_b = [np.ascontiguousarray(hidden_states[b].T) for b in range(B)]

    in_maps = []
    for core in range(NCORES):
        b, tq = core // 4, core % 4
        qoff = tq * SQ
        in_maps.append(
            {
                "hsT": hsT_b[b],
                "hsQ": np.ascontiguousarray(hsT_b[b][:, qoff : qoff + SQ]),
                "wq": Wq,
                "wk": Wk,
                "wv": Wv,
                "wo": Wo,
                "bqT": bqT_h,
                "bkT": bkT_h,
                "bv": bv_h,
                "cosq": np.ascontiguousarray(cosT[:, qoff : qoff + SQ]),
                "ssinq": np.ascontiguousarray(ssinT[:, qoff : qoff + SQ]),
                "cosk": cosT,
                "ssink": ssinT,
            }
        )
    return in_maps


def kernel(hidden_states, Wq, bq, Wk, bk, Wv, bv, Wo):
    from concourse.bass_utils import run_bass_kernel_spmd

    in_maps = _prepare_in_maps(hidden_states, Wq, bq, Wk, bk, Wv, bv, Wo)
    nc = _get_program()
    res = run_bass_kernel_spmd(
        nc, in_maps, core_ids=list(range(NCORES)), trace=False
    )

    out_full = np.empty((B, S, H), dtype=np.float32)
    for core in range(NCORES):
        b, tq = core // 4, core % 4
        out_full[b, tq * SQ : (tq + 1) * SQ, :] = res.results[core]["out"]
    return out_full



# revision 12
# speedup vs baseline: 1.2294x; 1.2294x over previous
"""Trainium2 Bass kernel for a GQA attention block (LuluAttention).

Problem: hidden_states [2, 2048, 2048], 16 q heads / 4 kv heads of dim 128,
RoPE, softmax attention, output projection.

Sharding: 8 cores = 2 (batch) x 4 (query-row blocks of 512 rows).
Each core computes the full K/V for its batch (all 4 kv heads), Q for its
512-row query slice (all 16 heads), RoPE, attention, and the output
projection for its row slice.  The full output is assembled on the host by
pure concatenation (no collectives needed).

Key implementation choices (vs a straightforward fp32 version):
  - All DMA'd matmul operands (hs, Wq, Wk, Wv, Wo) are bf16: 1 PE
    cycle/row (4x over fp32) and half the HBM traffic.  On-device
    intermediates (q/k/v/attn/ctx) are also bf16; PSUM accumulation stays
    fp32.  Measured end-to-end rel err ~5e-3 (gate is 2e-2).
  - hs^T is DMA'd once into SBUF and stays resident for the K and V
    projections (the fp32 version streamed it three times).
  - Everything is kept transposed ([head_dim, seq] with head_dim on SBUF
    partitions): QT/KT come straight out of matmul(lhsT=W_slice, rhs=hsT),
    scoresT = K @ Q^T, exp(scoresT) feeds the AV matmul directly
    (lhsT = V tile natural), denominator = ones @ expT, ctxT slices are
    directly the lhsT for the output projection.  No on-device transposes.
  - rotate_half for RoPE is a PE matmul with a constant +-1 permutation
    matrix R (rh = R^T @ x), replacing SBUF->SBUF half-swap DMAs.
  - Wo is prefetched into SBUF during the attention phase.
"""

import sys

if "/opt/trn_rl_repo" not in sys.path:
    sys.path.insert(0, "/opt/trn_rl_repo")

import numpy as np

B, S, H = 2, 2048, 2048
NH, NKV, D = 16, 4, 128
SQ = 512          # query rows per core
NCORES = 8
P = 128
NT = H // P       # 16 contraction tiles over hidden dim
ST = S // P       # 16 seq tiles
ROPE_THETA = 10000.0
SCALE = 1.0 / float(np.sqrt(D))
GROUPS = NH // NKV


def _np_bf16():
    from concourse import mybir

    return mybir.dt.np(mybir.dt.bfloat16)


def _rope_tables_T():
    """cosT/sinT [D, S]: transposed plain RoPE tables (the rotate-half sign
    lives in the R permutation matrix, not the tables)."""
    inv_freq = 1.0 / (ROPE_THETA ** (np.arange(0, D, 2, dtype=np.float64) / D))
    t = np.arange(S, dtype=np.float64)
    freqs = np.outer(t, inv_freq)                     # [S, D/2]
    emb = np.concatenate([freqs, freqs], axis=-1)     # [S, D]
    cos = np.cos(emb).astype(np.float32)
    sin = np.sin(emb).astype(np.float32)
    return np.ascontiguousarray(cos.T), np.ascontiguousarray(sin.T)


def _rotate_half_matrix():
    """R [128, 128] with rh = R^T @ x == rotate_half(x) for x [d, n]:
    rh[m] = -x[m+64] for m<64, rh[m] = x[m-64] for m>=64."""
    R = np.zeros((D, D), dtype=np.float32)
    for m in range(D // 2):
        R[m + D // 2, m] = -1.0
    for m in range(D // 2, D):
        R[m - D // 2, m] = 1.0
    return R


def _build_program():
    from concourse import bacc, mybir, tile

    F32 = mybir.dt.float32
    BF16 = mybir.dt.bfloat16
    AF = mybir.ActivationFunctionType

    nc = bacc.Bacc(
        "TRN2", target_bir_lowering=False, debug=False, num_devices=NCORES
    )

    hsT = nc.dram_tensor("hsT", [H, S], BF16, kind="ExternalInput").ap()
    hsQ = nc.dram_tensor("hsQ", [H, SQ], BF16, kind="ExternalInput").ap()
    wq = nc.dram_tensor("wq", [H, NH * D], BF16, kind="ExternalInput").ap()
    wk = nc.dram_tensor("wk", [H, NKV * D], BF16, kind="ExternalInput").ap()
    wv = nc.dram_tensor("wv", [H, NKV * D], BF16, kind="ExternalInput").ap()
    wo = nc.dram_tensor("wo", [NH * D, H], BF16, kind="ExternalInput").ap()
    bqT = nc.dram_tensor("bqT", [D, NH], F32, kind="ExternalInput").ap()
    bkT = nc.dram_tensor("bkT", [D, NKV], F32, kind="ExternalInput").ap()
    bv = nc.dram_tensor("bv", [1, NKV * D], BF16, kind="ExternalInput").ap()
    rmat = nc.dram_tensor("rmat", [D, D], BF16, kind="ExternalInput").ap()
    cosq = nc.dram_tensor("cosq", [D, SQ], F32, kind="ExternalInput").ap()
    sinq = nc.dram_tensor("sinq", [D, SQ], F32, kind="ExternalInput").ap()
    cosk = nc.dram_tensor("cosk", [D, S], F32, kind="ExternalInput").ap()
    sink = nc.dram_tensor("sink", [D, S], F32, kind="ExternalInput").ap()
    out = nc.dram_tensor("out", [SQ, H], F32, kind="ExternalOutput").ap()

    with tile.TileContext(nc) as tc:
        # ---- long-lived left-side pools ----
        cst = tc.alloc_tile_pool(name="cst", bufs=1)
        kvq = tc.alloc_tile_pool(name="kvq", bufs=1)
        ctxp = tc.alloc_tile_pool(name="ctxp", bufs=1)

        ones1 = cst.tile([1, P], BF16, tag="ones1")
        nc.gpsimd.memset(ones1[:], 1.0)
        ones128 = cst.tile([P, P], BF16, tag="ones128")
        nc.gpsimd.memset(ones128[:], 1.0)
        bqT_sb = cst.tile([D, NH], F32, tag="bqT")
        nc.sync.dma_start(bqT_sb[:], bqT[:, :])
        bkT_sb = cst.tile([D, NKV], F32, tag="bkT")
        nc.sync.dma_start(bkT_sb[:], bkT[:, :])
        bv_sb = cst.tile([1, NKV * D], BF16, tag="bv")
        nc.sync.dma_start(bv_sb[:], bv[:, :])
        r_sb = cst.tile([D, D], BF16, tag="rmat")
        nc.sync.dma_start(r_sb[:], rmat[:, :])
        cosq_sb = cst.tile([D, SQ], F32, tag="cosq")
        sinq_sb = cst.tile([D, SQ], F32, tag="sinq")
        cosk_sb = cst.tile([D, S], F32, tag="cosk")
        sink_sb = cst.tile([D, S], F32, tag="sink")

        # persistent bf16 intermediates
        qt = [kvq.tile([D, SQ], BF16, tag=f"qt{h}", name=f"qt{h}") for h in range(NH)]
        kt = [kvq.tile([D, S], BF16, tag=f"kt{g}", name=f"kt{g}") for g in range(NKV)]
        vt = [kvq.tile([P, NKV * D], BF16, tag=f"v{t}", name=f"v{t}") for t in range(ST)]
        ctx = [ctxp.tile([D, SQ], BF16, tag=f"ctx{h}", name=f"ctx{h}") for h in range(NH)]

        # ---- phase 1: projections (Q first: it needs only hsQ+wq DMA,
        # so the PE starts early while the big hsT load streams in).
        # RoPE is interleaved right behind each drained projection chunk so
        # the DVE work overlaps the remaining projection matmuls. ----
        hsp = tc.alloc_tile_pool(name="hsp", bufs=1, side="right")
        ws1 = tc.alloc_tile_pool(name="ws1", bufs=1, side="right")
        hsq = [hsp.tile([P, SQ], BF16, tag=f"hsq{t}", name=f"hsq{t}") for t in range(NT)]
        for ht in range(NT):
            nc.sync.dma_start(hsq[ht][:], hsQ[ht * P : (ht + 1) * P, :])
        hst = [hsp.tile([P, S], BF16, tag=f"hst{t}", name=f"hst{t}") for t in range(NT)]

        ps6 = tc.alloc_tile_pool(name="ps6", bufs=6, space="PSUM")
        rhp = tc.alloc_tile_pool(name="rhp", bufs=2, space="PSUM")
        rsc = tc.alloc_tile_pool(name="rsc", bufs=2, side="right")

        def rope(dst, tbl_cos, tbl_sin):
            """dst [128, 512] bf16, in place; rh = R^T @ x on the PE."""
            rh = rhp.tile([P, 512], F32, tag="rh", name="rh")
            nc.tensor.matmul(rh[:], r_sb[:], dst[:], start=True, stop=True)
            t1 = rsc.tile([P, 512], F32, tag="rt1", bufs=2, name="rt1")
            nc.vector.tensor_mul(t1[:], rh[:], tbl_sin)
            t2 = rsc.tile([P, 512], F32, tag="rt2", bufs=2, name="rt2")
            nc.vector.tensor_mul(t2[:], dst[:], tbl_cos)
            nc.vector.tensor_add(dst[:], t1[:], t2[:])

        # -- Q: three sweeps of <=6 heads; wq streamed in per-sweep chunks --
        for h0, h1 in ((0, 6), (6, 12), (12, 16)):
            banks = {
                h: ps6.tile([P, SQ], F32, tag="pj", name=f"pq{h}")
                for h in range(h0, h1)
            }
            for ht in range(NT):
                wqt = ws1.tile(
                    [P, 6 * P], BF16, tag="wq", bufs=3, name="wqt"
                )
                nc.sync.dma_start(
                    wqt[:, : (h1 - h0) * P],
                    wq[ht * P : (ht + 1) * P, h0 * D : h1 * D],
                )
                for h in range(h0, h1):
                    nc.tensor.matmul(
                        banks[h][:],
                        wqt[:, (h - h0) * P : (h - h0 + 1) * P],
                        hsq[ht][:],
                        start=(ht == 0),
                        stop=(ht == NT - 1),
                    )
            if h0 == 0:
                # tables + hsT stream while the first Q sweep computes
                nc.sync.dma_start(cosq_sb[:], cosq[:, :])
                nc.sync.dma_start(sinq_sb[:], sinq[:, :])
                nc.sync.dma_start(cosk_sb[:], cosk[:, :])
                nc.sync.dma_start(sink_sb[:], sink[:, :])
                for ht in range(NT):
                    nc.sync.dma_start(
                        hst[ht][:], hsT[ht * P : (ht + 1) * P, :]
                    )
            for h in range(h0, h1):
                nc.scalar.activation(
                    qt[h][:], banks[h][:], AF.Identity,
                    bias=bqT_sb[:, h : h + 1],
                )
                rope(qt[h][:], cosq_sb[:], sinq_sb[:])

        # -- K: one kv head (4 seq chunks) per sweep; kt transposed --
        for g in range(NKV):
            banks = {
                c: ps6.tile([P, 512], F32, tag="pj", name=f"pk{g}_{c}")
                for c in range(4)
            }
            for ht in range(NT):
                wkt = ws1.tile([P, D], BF16, tag="wk", bufs=4, name="wkt")
                nc.sync.dma_start(
                    wkt[:],
                    wk[ht * P : (ht + 1) * P, g * D : (g + 1) * D],
                )
                for c in range(4):
                    nc.tensor.matmul(
                        banks[c][:],
                        wkt[:],
                        hst[ht][:, c * 512 : (c + 1) * 512],
                        start=(ht == 0),
                        stop=(ht == NT - 1),
                    )
            for c in range(4):
                nc.scalar.activation(
                    kt[g][:, c * 512 : (c + 1) * 512],
                    banks[c][:],
                    AF.Identity,
                    bias=bkT_sb[:, g : g + 1],
                )
                rope(
                    kt[g][:, c * 512 : (c + 1) * 512],
                    cosk_sb[:, c * 512 : (c + 1) * 512],
                    sink_sb[:, c * 512 : (c + 1) * 512],
                )

        # -- V: natural layout, lhsT = resident hsT column slices --
        wvres = [
            ws1.tile([P, NKV * D], BF16, tag=f"wv{t}", name=f"wv{t}")
            for t in range(NT)
        ]
        for ht in range(NT):
            nc.sync.dma_start(wvres[ht][:], wv[ht * P : (ht + 1) * P, :])
        for t in range(ST):
            bank = ps6.tile([P, 512], F32, tag="pj", name=f"pv{t}")
            for ht in range(NT):
                nc.tensor.matmul(
                    bank[:],
                    hst[ht][:, t * P : (t + 1) * P],
                    wvres[ht][:],
                    start=(ht == 0),
                    stop=False,
                )
            nc.tensor.matmul(
                bank[:], ones1[:], bv_sb[:], start=False, stop=True
            )
            nc.scalar.copy(vt[t][:], bank[:])
        rsc.release()
        rhp.release()
        ps6.release()
        ws1.release()
        hsp.release()

        # ---- phase 2: attention; wo prefetch streams during it ----
        wop = tc.alloc_tile_pool(name="wop", bufs=1, side="right")
        wores = [
            wop.tile([P, H], BF16, tag=f"wo{h}", name=f"wo{h}")
            for h in range(NH)
        ]
        for h in range(NH):
            nc.sync.dma_start(wores[h][:], wo[h * D : (h + 1) * D, :])

        # out_acc[s1t] [128, 2048] f32 accumulates the output projection in
        # SBUF; groups of 4 heads go PSUM -> (DVE add) -> out_acc, so the
        # out-proj matmuls ride the PE's slack in the Act-bound attention
        # phase and there is no separate phase 3.
        oacc = tc.alloc_tile_pool(name="oacc", bufs=1, side="right")
        out_acc = [
            oacc.tile([P, H], F32, tag=f"oa{s}", name=f"oa{s}")
            for s in range(4)
        ]

        pa = tc.alloc_tile_pool(name="ps_a", bufs=3, space="PSUM")
        pc = tc.alloc_tile_pool(name="ps_c", bufs=3, space="PSUM")
        pob = tc.alloc_tile_pool(name="ps_ob", bufs=2, space="PSUM")
        wsa = tc.alloc_tile_pool(name="wsa", bufs=4, side="right")
        DG = 8  # den group: sum DG exp-tiles on the DVE, 1 den matmul/group

        # Deferred out-proj emitters: each is one PE matmul (or one DVE
        # drain); they are popped one per (h, t) step so they fill the PE's
        # slack without ever blocking the Act-engine exp stream.
        pending = []

        def push_group_out(h):
            """Queue out-proj work for finished heads h-3..h."""
            for hc in range(4):
                for s1t in range(4):
                    bank_box = []

                    def mk(hh, hc=hc, s1t=s1t, h=h, bank_box=bank_box):
                        def emit():
                            if not bank_box:
                                bank_box.append(
                                    pob.tile(
                                        [P, 512], F32, tag="po", name="po"
                                    )
                                )
                            nc.tensor.matmul(
                                bank_box[0][:],
                                ctx[hh][:, s1t * P : (s1t + 1) * P],
                                wores[hh][:, hc * 512 : (hc + 1) * 512],
                                start=(hh == h - 3),
                                stop=(hh == h),
                            )

                        return emit

                    for hh in range(h - 3, h + 1):
                        emit = mk(hh)
                        emit.is_mm = True
                        pending.append(emit)

                    def drain(hc=hc, s1t=s1t, h=h, bank_box=bank_box):
                        dst = out_acc[s1t][:, hc * 512 : (hc + 1) * 512]
                        if h == 3:
                            nc.vector.tensor_copy(dst, bank_box[0][:])
                        else:
                            nc.vector.tensor_add(dst, dst, bank_box[0][:])

                    pending.append(drain)

        def pop_pending():
            """Emit queued DVE drains freely plus one PE matmul."""
            while pending:
                fn = pending.pop(0)
                fn()
                if getattr(fn, "is_mm", False):
                    break

        for h in range(NH):
            g = h // GROUPS
            ctx_ps = pc.tile([P, SQ], F32, tag="acc", name=f"ctxps{h}")
            den_ps = pc.tile([P, SQ], F32, tag="acc", name=f"denps{h}")
            sc = [None] * ST

            def score(t):
                sc[t] = pa.tile([P, SQ], F32, tag="sc", name="sc")
                nc.tensor.matmul(
                    sc[t][:],
                    kt[g][:, t * P : (t + 1) * P],
                    qt[h][:],
                    start=True,
                    stop=True,
                )

            score(0)
            asum = None
            for t in range(ST):
                at = wsa.tile([P, SQ], BF16, tag="at", name="at")
                nc.scalar.activation(at[:], sc[t][:], AF.Exp, scale=SCALE)
                if t + 1 < ST:
                    score(t + 1)
                nc.tensor.matmul(
                    ctx_ps[:],
                    vt[t][:, g * D : (g + 1) * D],
                    at[:],
                    start=(t == 0),
                    stop=(t == ST - 1),
                )
                if t % DG == 0:
                    at0 = at
                elif t % DG == 1:
                    asum = wsa.tile([P, SQ], BF16, tag="as", bufs=2, name="asum")
                    nc.vector.tensor_add(asum[:], at0[:], at[:])
                else:
                    nc.vector.tensor_add(asum[:], asum[:], at[:])
                if t % DG == DG - 1:
                    nc.tensor.matmul(
                        den_ps[:],
                        ones128[:],
                        asum[:],
                        start=(t == DG - 1),
                        stop=(t == ST - 1),
                    )
                pop_pending()
            rc = wsa.tile([P, SQ], F32, tag="rc", bufs=2, name="rc")
            nc.vector.reciprocal(rc[:], den_ps[:])
            nc.vector.tensor_mul(ctx[h][:], ctx_ps[:], rc[:])

            if h % 4 == 3:
                push_group_out(h)
        while pending:
            pending.pop(0)()
        for s1t in range(4):
            nc.sync.dma_start(out[s1t * P : (s1t + 1) * P, :], out_acc[s1t][:])
        wsa.release()
        pob.release()
        pc.release()
        pa.release()
        oacc.release()
        wop.release()
        ctxp.release()
        kvq.release()
        cst.release()

    nc.compile()
    return nc


_PROGRAM_CACHE = {}


def _get_program():
    if "nc" not in _PROGRAM_CACHE:
        _PROGRAM_CACHE["nc"] = _build_program()
    return _PROGRAM_CACHE["nc"]


def _prepare_in_maps(hidden_states, Wq, bq, Wk, bk, Wv, bv, Wo):
    bf16 = _np_bf16()
    hidden_states = np.asarray(hidden_states, dtype=np.float32)
    Wq_b = np.asarray(Wq, dtype=np.float32).astype(bf16)
    Wk_b = np.asarray(Wk, dtype=np.float32).astype(bf16)
    Wv_b = np.asarray(Wv, dtype=np.float32).astype(bf16)
    Wo_b = np.asarray(Wo, dtype=np.float32).astype(bf16)
    bq = np.asarray(bq, dtype=np.float32)
    bk = np.asarray(bk, dtype=np.float32)
    bv_b = np.asarray(bv, dtype=np.float32).astype(bf16).reshape(1, NKV * D)

    cosT, sinT = _rope_tables_T()
    rmat = _rotate_half_matrix().astype(bf16)
    bqT_h = np.ascontiguousarray(bq.reshape(NH, D).T)    # [128, 16]
    bkT_h = np.ascontiguousarray(bk.reshape(NKV, D).T)   # [128, 4]

    hsT_b = [
        np.ascontiguousarray(hidden_states[b].T).astype(bf16) for b in range(B)
    ]

    in_maps = []
    for core in range(NCORES):
        b, tq = core // 4, core % 4
        qoff = tq * SQ
        in_maps.append(
            {
                "hsT": hsT_b[b],
                "hsQ": np.ascontiguousarray(hsT_b[b][:, qoff : qoff + SQ]),
                "wq": Wq_b,
                "wk": Wk_b,
                "wv": Wv_b,
                "wo": Wo_b,
                "bqT": bqT_h,
                "bkT": bkT_h,
                "bv": bv_b,
                "rmat": rmat,
                "cosq": np.ascontiguousarray(cosT[:, qoff : qoff + SQ]),
                "sinq": np.ascontiguousarray(sinT[:, qoff : qoff + SQ]),
                "cosk": cosT,
                "sink": sinT,
            }
        )
    return in_maps


def kernel(hidden_states, Wq, bq, Wk, bk, Wv, bv, Wo):
    from concourse.bass_utils import run_bass_kernel_spmd

    in_maps = _prepare_in_maps(hidden_states, Wq, bq, Wk, bk, Wv, bv, Wo)
    nc = _get_program()
    res = run_bass_kernel_spmd(
        nc, in_maps, core_ids=list(range(NCORES)), trace=False
    )

    out_full = np.empty((B, S, H), dtype=np.float32)
    for core in range(NCORES):
        b, tq = core // 4, core % 4
        out_full[b, tq * SQ : (tq + 1) * SQ, :] = res.results[core]["out"]
    return out_full


# revision 18
# speedup vs baseline: 2.0110x; 1.6358x over previous
"""Trainium2 Bass kernel for a GQA attention block (LuluAttention).

Problem: hidden_states [2, 2048, 2048], 16 q heads / 4 kv heads of dim 128,
RoPE, softmax attention, output projection.

Sharding: 8 cores = 2 (batch) x 4 (query-row blocks of 512 rows).
Each core computes the full K/V for its batch (all 4 kv heads), Q for its
512-row query slice (all 16 heads), RoPE, attention, and the output
projection for its row slice.  The full output is assembled on the host by
pure concatenation (no collectives needed).

Key implementation choices (vs a straightforward fp32 version):
  - All DMA'd matmul operands (hs, Wq, Wk, Wv, Wo) are bf16: 1 PE
    cycle/row (4x over fp32) and half the HBM traffic.  On-device
    intermediates (q/k/v/attn/ctx) are also bf16; PSUM accumulation stays
    fp32.  Measured end-to-end rel err ~5e-3 (gate is 2e-2).
  - hs^T is DMA'd once into SBUF and stays resident for the K and V
    projections (the fp32 version streamed it three times).
  - Everything is kept transposed ([head_dim, seq] with head_dim on SBUF
    partitions): QT/KT come straight out of matmul(lhsT=W_slice, rhs=hsT),
    scoresT = K @ Q^T, exp(scoresT) feeds the AV matmul directly
    (lhsT = V tile natural), denominator = ones @ expT, ctxT slices are
    directly the lhsT for the output projection.  No on-device transposes.
  - rotate_half for RoPE is a PE matmul with a constant +-1 permutation
    matrix R (rh = R^T @ x), replacing SBUF->SBUF half-swap DMAs.
  - Wo is prefetched into SBUF during the attention phase.
"""

import sys

if "/opt/trn_rl_repo" not in sys.path:
    sys.path.insert(0, "/opt/trn_rl_repo")

import numpy as np

B, S, H = 2, 2048, 2048
NH, NKV, D = 16, 4, 128
SQ = 512          # query rows per core
NCORES = 8
P = 128
NT = H // P       # 16 contraction tiles over hidden dim
ST = S // P       # 16 seq tiles
ROPE_THETA = 10000.0
SCALE = 1.0 / float(np.sqrt(D))
GROUPS = NH // NKV


def _np_bf16():
    from concourse import mybir

    return mybir.dt.np(mybir.dt.bfloat16)


def _rope_tables_T():
    """cosT/sinT [D, S]: transposed plain RoPE tables (the rotate-half sign
    lives in the R permutation matrix, not the tables)."""
    inv_freq = 1.0 / (ROPE_THETA ** (np.arange(0, D, 2, dtype=np.float64) / D))
    t = np.arange(S, dtype=np.float64)
    freqs = np.outer(t, inv_freq)                     # [S, D/2]
    emb = np.concatenate([freqs, freqs], axis=-1)     # [S, D]
    cos = np.cos(emb).astype(np.float32)
    sin = np.sin(emb).astype(np.float32)
    return np.ascontiguousarray(cos.T), np.ascontiguousarray(sin.T)


def _rotate_half_matrix():
    """R [128, 128] with rh = R^T @ x == rotate_half(x) for x [d, n]:
    rh[m] = -x[m+64] for m<64, rh[m] = x[m-64] for m>=64."""
    R = np.zeros((D, D), dtype=np.float32)
    for m in range(D // 2):
        R[m + D // 2, m] = -1.0
    for m in range(D // 2, D):
        R[m - D // 2, m] = 1.0
    return R


def _build_program():
    from concourse import bacc, mybir, tile

    F32 = mybir.dt.float32
    BF16 = mybir.dt.bfloat16
    AF = mybir.ActivationFunctionType

    nc = bacc.Bacc(
        "TRN2", target_bir_lowering=False, debug=False, num_devices=NCORES
    )

    # big operands come in host-tiled [128, n*cols] layouts so each loads
    # with a single large DMA: x_t[p, i*cols + c] = x[i*128 + p, c]
    hsT = nc.dram_tensor("hsT", [P, NT * S], BF16, kind="ExternalInput").ap()
    hsQ = nc.dram_tensor("hsQ", [P, NT * SQ], BF16, kind="ExternalInput").ap()
    wq = nc.dram_tensor("wq", [P, NT * NH * D], BF16, kind="ExternalInput").ap()
    wk = nc.dram_tensor("wk", [P, NT * NKV * D], BF16, kind="ExternalInput").ap()
    wv = nc.dram_tensor("wv", [P, NT * NKV * D], BF16, kind="ExternalInput").ap()
    wo = nc.dram_tensor("wo", [P, NH * H], BF16, kind="ExternalInput").ap()
    bqT = nc.dram_tensor("bqT", [D, NH], F32, kind="ExternalInput").ap()
    bkT = nc.dram_tensor("bkT", [D, NKV], F32, kind="ExternalInput").ap()
    bv = nc.dram_tensor("bv", [1, NKV * D], BF16, kind="ExternalInput").ap()
    rmat = nc.dram_tensor("rmat", [D, D], BF16, kind="ExternalInput").ap()
    cosq = nc.dram_tensor("cosq", [D, SQ], F32, kind="ExternalInput").ap()
    sinq = nc.dram_tensor("sinq", [D, SQ], F32, kind="ExternalInput").ap()
    cosk = nc.dram_tensor("cosk", [D, S], F32, kind="ExternalInput").ap()
    sink = nc.dram_tensor("sink", [D, S], F32, kind="ExternalInput").ap()
    out = nc.dram_tensor("out", [SQ, H], F32, kind="ExternalOutput").ap()

    with tile.TileContext(nc) as tc:
        # ---- long-lived left-side pools ----
        cst = tc.alloc_tile_pool(name="cst", bufs=1)
        kvq = tc.alloc_tile_pool(name="kvq", bufs=1)
        ctxp = tc.alloc_tile_pool(name="ctxp", bufs=1)

        ones1 = cst.tile([1, P], BF16, tag="ones1")
        nc.gpsimd.memset(ones1[:], 1.0)
        ones128 = cst.tile([P, P], BF16, tag="ones128")
        nc.gpsimd.memset(ones128[:], 1.0)
        bqT_sb = cst.tile([D, NH], F32, tag="bqT")
        nc.scalar.dma_start(bqT_sb[:], bqT[:, :])
        bkT_sb = cst.tile([D, NKV], F32, tag="bkT")
        nc.scalar.dma_start(bkT_sb[:], bkT[:, :])
        bv_sb = cst.tile([1, NKV * D], BF16, tag="bv")
        nc.scalar.dma_start(bv_sb[:], bv[:, :])
        r_sb = cst.tile([D, D], BF16, tag="rmat")
        nc.scalar.dma_start(r_sb[:], rmat[:, :])
        cosq_sb = cst.tile([D, SQ], F32, tag="cosq")
        sinq_sb = cst.tile([D, SQ], F32, tag="sinq")
        cosk_sb = cst.tile([D, S], F32, tag="cosk")
        sink_sb = cst.tile([D, S], F32, tag="sink")

        # persistent bf16 intermediates
        qt = [kvq.tile([D, SQ], BF16, tag=f"qt{h}", name=f"qt{h}") for h in range(NH)]
        kt = [kvq.tile([D, S], BF16, tag=f"kt{g}", name=f"kt{g}") for g in range(NKV)]
        vt = [kvq.tile([P, NKV * D], BF16, tag=f"v{t}", name=f"v{t}") for t in range(ST)]
        ctx = [ctxp.tile([D, SQ], BF16, tag=f"ctx{h}", name=f"ctx{h}") for h in range(NH)]

        # ---- phase 1: projections (Q first: it needs only hsQ+wq DMA,
        # so the PE starts early while the big hsT load streams in).
        # RoPE is interleaved right behind each drained projection chunk so
        # the DVE work overlaps the remaining projection matmuls. ----
        hsp = tc.alloc_tile_pool(name="hsp", bufs=1, side="right")
        ws1 = tc.alloc_tile_pool(name="ws1", bufs=1, side="right")
        hsq = hsp.tile([P, NT * SQ], BF16, tag="hsq", name="hsq")
        hst = hsp.tile([P, NT * S], BF16, tag="hst", name="hst")

        # One PSUM pool set for ALL phases (3+3+2 banks): re-allocating
        # pools at a phase boundary coalesces the WAR dependency into an
        # engine-counter barrier over the whole previous phase.
        pa = tc.alloc_tile_pool(name="ps_a", bufs=3, space="PSUM")
        pc = tc.alloc_tile_pool(name="ps_c", bufs=3, space="PSUM")
        pob = tc.alloc_tile_pool(name="ps_ob", bufs=2, space="PSUM")
        rsc = tc.alloc_tile_pool(name="rsc", bufs=2, side="right")

        def bank6(i, name):
            """Alternate pa/pc so up to 6 accumulators are live at once."""
            pool = pa if i % 2 == 0 else pc
            return pool.tile([P, 512], F32, tag="pj", name=name)

        def rope(dst, tbl_cos, tbl_sin):
            """dst [128, 512] bf16, in place; rh = R^T @ x on the PE."""
            rh = pob.tile([P, 512], F32, tag="po", name="rh")
            nc.tensor.matmul(rh[:], r_sb[:], dst[:], start=True, stop=True)
            t1 = rsc.tile([P, 512], F32, tag="rt1", bufs=2, name="rt1")
            nc.vector.tensor_mul(t1[:], rh[:], tbl_sin)
            t2 = rsc.tile([P, 512], F32, tag="rt2", bufs=2, name="rt2")
            nc.vector.tensor_mul(t2[:], dst[:], tbl_cos)
            nc.vector.tensor_add(dst[:], t1[:], t2[:])

        # -- Q: three sweeps of <=6 heads; wq streamed in per-sweep chunks --
        for h0, h1 in ((0, 6), (6, 12), (12, 16)):
            banks = {
                h: bank6(h - h0, f"pq{h}") for h in range(h0, h1)
            }
            for ht in range(NT):
                if h0 == 0:
                    nc.sync.dma_start(
                        hsq[:, ht * SQ : (ht + 1) * SQ],
                        hsQ[:, ht * SQ : (ht + 1) * SQ],
                    )
                wqt = ws1.tile(
                    [P, 6 * P], BF16, tag="wq", bufs=3, name="wqt"
                )
                nc.sync.dma_start(
                    wqt[:, : (h1 - h0) * P],
                    wq[:, ht * NH * D + h0 * D : ht * NH * D + h1 * D],
                )
                for h in range(h0, h1):
                    nc.tensor.matmul(
                        banks[h][:],
                        wqt[:, (h - h0) * P : (h - h0 + 1) * P],
                        hsq[:, ht * SQ : (ht + 1) * SQ],
                        start=(ht == 0),
                        stop=(ht == NT - 1),
                    )
            if h0 == 0:
                # q-rope tables stream during sweep 1; the hsT bulk waits
                # until sweep 2 so it never contends with the critical
                # hsq/wq stream (K first touches hsT much later)
                nc.scalar.dma_start(cosq_sb[:], cosq[:, :])
                nc.scalar.dma_start(sinq_sb[:], sinq[:, :])
            if h0 == 6:
                for i in range(8):
                    cs = NT * S // 8
                    nc.scalar.dma_start(
                        hst[:, i * cs : (i + 1) * cs],
                        hsT[:, i * cs : (i + 1) * cs],
                    )
                nc.scalar.dma_start(cosk_sb[:], cosk[:, :])
                nc.scalar.dma_start(sink_sb[:], sink[:, :])
            for h in range(h0, h1):
                nc.scalar.activation(
                    qt[h][:], banks[h][:], AF.Identity,
                    bias=bqT_sb[:, h : h + 1],
                )
                rope(qt[h][:], cosq_sb[:], sinq_sb[:])

        # -- K: one kv head (4 seq chunks) per sweep; kt transposed --
        for g in range(NKV):
            banks = {c: bank6(c, f"pk{g}_{c}") for c in range(4)}
            for ht in range(NT):
                wkt = ws1.tile([P, D], BF16, tag="wk", bufs=4, name="wkt")
                nc.sync.dma_start(
                    wkt[:],
                    wk[:, ht * NKV * D + g * D : ht * NKV * D + (g + 1) * D],
                )
                for c in range(4):
                    nc.tensor.matmul(
                        banks[c][:],
                        wkt[:],
                        hst[:, ht * S + c * 512 : ht * S + (c + 1) * 512],
                        start=(ht == 0),
                        stop=(ht == NT - 1),
                    )
            for c in range(4):
                nc.scalar.activation(
                    kt[g][:, c * 512 : (c + 1) * 512],
                    banks[c][:],
                    AF.Identity,
                    bias=bkT_sb[:, g : g + 1],
                )
                rope(
                    kt[g][:, c * 512 : (c + 1) * 512],
                    cosk_sb[:, c * 512 : (c + 1) * 512],
                    sink_sb[:, c * 512 : (c + 1) * 512],
                )

        # -- V: natural layout, lhsT = resident hsT column slices --
        wvres = ws1.tile([P, NT * NKV * D], BF16, tag="wv", name="wvres")
        for i in range(4):
            q = NT * NKV * D // 4
            nc.scalar.dma_start(
                wvres[:, i * q : (i + 1) * q], wv[:, i * q : (i + 1) * q]
            )
        for t in range(ST):
            bank = bank6(t, f"pv{t}")
            for ht in range(NT):
                nc.tensor.matmul(
                    bank[:],
                    hst[:, ht * S + t * P : ht * S + (t + 1) * P],
                    wvres[:, ht * NKV * D : (ht + 1) * NKV * D],
                    start=(ht == 0),
                    stop=False,
                )
            nc.tensor.matmul(
                bank[:], ones1[:], bv_sb[:], start=False, stop=True
            )
            nc.scalar.copy(vt[t][:], bank[:])
        rsc.release()
        ws1.release()
        hsp.release()

        # ---- phase 2: attention; wo prefetch streams during it ----
        wop = tc.alloc_tile_pool(name="wop", bufs=1, side="right")
        wores = wop.tile([P, NH * H], BF16, tag="wo", name="wores")
        for i in range(4):
            cs = NH * H // 4
            nc.sync.dma_start(
                wores[:, i * cs : (i + 1) * cs], wo[:, i * cs : (i + 1) * cs]
            )

        # out_acc[s1t] [128, 2048] f32 accumulates the output projection in
        # SBUF; groups of 4 heads go PSUM -> (DVE add) -> out_acc, so the
        # out-proj matmuls ride the PE's slack in the Act-bound attention
        # phase and there is no separate phase 3.
        oacc = tc.alloc_tile_pool(name="oacc", bufs=1, side="right")
        out_acc = [
            oacc.tile([P, H], F32, tag=f"oa{s}", name=f"oa{s}")
            for s in range(4)
        ]

        wsa = tc.alloc_tile_pool(name="wsa", bufs=4, side="right")
        DG = 8  # den group: sum DG exp-tiles on the DVE, 1 den matmul/group

        # Deferred out-proj emitters: each is one PE matmul (or one DVE
        # drain); they are popped one per (h, t) step so they fill the PE's
        # slack without ever blocking the Act-engine exp stream.
        pending = []

        def push_group_out(h):
            """Queue out-proj work for finished heads h-3..h."""
            for s1t in range(4):
                for hc in range(4):
                    bank_box = []

                    def mk(hh, hc=hc, s1t=s1t, h=h, bank_box=bank_box):
                        def emit():
                            if not bank_box:
                                bank_box.append(
                                    pob.tile(
                                        [P, 512], F32, tag="po", name="po"
                                    )
                                )
                            nc.tensor.matmul(
                                bank_box[0][:],
                                ctx[hh][:, s1t * P : (s1t + 1) * P],
                                wores[
                                    :,
                                    hh * H + hc * 512 : hh * H + (hc + 1) * 512,
                                ],
                                start=(hh == h - 3),
                                stop=(hh == h),
                            )

                        return emit

                    for hh in range(h - 3, h + 1):
                        emit = mk(hh)
                        emit.is_mm = True
                        pending.append(emit)

                    def drain(hc=hc, s1t=s1t, h=h, bank_box=bank_box):
                        dst = out_acc[s1t][:, hc * 512 : (hc + 1) * 512]
                        if h == 3:
                            nc.vector.tensor_copy(dst, bank_box[0][:])
                        else:
                            nc.vector.tensor_add(dst, dst, bank_box[0][:])

                    pending.append(drain)
                    if h == NH - 1:
                        def outdma(s1t=s1t, hc=hc):
                            nc.sync.dma_start(
                                out[
                                    s1t * P : (s1t + 1) * P,
                                    hc * 512 : (hc + 1) * 512,
                                ],
                                out_acc[s1t][:, hc * 512 : (hc + 1) * 512],
                            )

                        pending.append(outdma)

        def pop_pending():
            """Emit queued DVE drains freely plus one PE matmul."""
            while pending:
                fn = pending.pop(0)
                fn()
                if getattr(fn, "is_mm", False):
                    break

        for h in range(NH):
            g = h // GROUPS
            ctx_ps = pc.tile([P, SQ], F32, tag="pj", name=f"ctxps{h}")
            den_ps = pc.tile([P, SQ], F32, tag="pj", name=f"denps{h}")
            sc = [None] * ST

            def score(t):
                sc[t] = pa.tile([P, SQ], F32, tag="pj", name="sc")
                nc.tensor.matmul(
                    sc[t][:],
                    kt[g][:, t * P : (t + 1) * P],
                    qt[h][:],
                    start=True,
                    stop=True,
                )

            score(0)
            asum = None
            for t in range(ST):
                at = wsa.tile([P, SQ], BF16, tag="at", name="at")
                nc.scalar.activation(at[:], sc[t][:], AF.Exp, scale=SCALE)
                if t + 1 < ST:
                    score(t + 1)
                nc.tensor.matmul(
                    ctx_ps[:],
                    vt[t][:, g * D : (g + 1) * D],
                    at[:],
                    start=(t == 0),
                    stop=(t == ST - 1),
                )
                if t % DG == 0:
                    at0 = at
                elif t % DG == 1:
                    asum = wsa.tile([P, SQ], BF16, tag="as", bufs=2, name="asum")
                    nc.vector.tensor_add(asum[:], at0[:], at[:])
                else:
                    nc.vector.tensor_add(asum[:], asum[:], at[:])
                if t % DG == DG - 1:
                    nc.tensor.matmul(
                        den_ps[:],
                        ones128[:],
                        asum[:],
                        start=(t == DG - 1),
                        stop=(t == ST - 1),
                    )
                pop_pending()
            rc = wsa.tile([P, SQ], F32, tag="rc", bufs=2, name="rc")
            nc.vector.reciprocal(rc[:], den_ps[:])
            nc.vector.tensor_mul(ctx[h][:], ctx_ps[:], rc[:])

            if h % 4 == 3:
                push_group_out(h)
        while pending:
            pending.pop(0)()
        wsa.release()
        oacc.release()
        wop.release()
        pob.release()
        pc.release()
        pa.release()
        ctxp.release()
        kvq.release()
        cst.release()

    nc.compile()
    return nc


_PROGRAM_CACHE = {}


def _get_program():
    if "nc" not in _PROGRAM_CACHE:
        _PROGRAM_CACHE["nc"] = _build_program()
    return _PROGRAM_CACHE["nc"]


def _tile_rows(x_b):
    """[n*128, cols] -> [128, n*cols] with x_t[p, i*cols + c] = x[i*128+p, c]."""
    n = x_b.shape[0] // P
    cols = x_b.shape[1]
    return np.ascontiguousarray(
        x_b.reshape(n, P, cols).transpose(1, 0, 2).reshape(P, n * cols)
    )


def _prepare_in_maps(hidden_states, Wq, bq, Wk, bk, Wv, bv, Wo):
    bf16 = _np_bf16()
    hidden_states = np.asarray(hidden_states, dtype=np.float32)
    Wq_t = _tile_rows(np.asarray(Wq, dtype=np.float32).astype(bf16))
    Wk_t = _tile_rows(np.asarray(Wk, dtype=np.float32).astype(bf16))
    Wv_t = _tile_rows(np.asarray(Wv, dtype=np.float32).astype(bf16))
    Wo_t = _tile_rows(np.asarray(Wo, dtype=np.float32).astype(bf16))
    bq = np.asarray(bq, dtype=np.float32)
    bk = np.asarray(bk, dtype=np.float32)
    bv_b = np.asarray(bv, dtype=np.float32).astype(bf16).reshape(1, NKV * D)

    cosT, sinT = _rope_tables_T()
    rmat = _rotate_half_matrix().astype(bf16)
    bqT_h = np.ascontiguousarray(bq.reshape(NH, D).T)    # [128, 16]
    bkT_h = np.ascontiguousarray(bk.reshape(NKV, D).T)   # [128, 4]

    hsT_b = [
        np.ascontiguousarray(hidden_states[b].T).astype(bf16) for b in range(B)
    ]
    hsT_t = [_tile_rows(h) for h in hsT_b]

    in_maps = []
    for core in range(NCORES):
        b, tq = core // 4, core % 4
        qoff = tq * SQ
        in_maps.append(
            {
                "hsT": hsT_t[b],
                "hsQ": _tile_rows(
                    np.ascontiguousarray(hsT_b[b][:, qoff : qoff + SQ])
                ),
                "wq": Wq_t,
                "wk": Wk_t,
                "wv": Wv_t,
                "wo": Wo_t,
                "bqT": bqT_h,
                "bkT": bkT_h,
                "bv": bv_b,
                "rmat": rmat,
                "cosq": np.ascontiguousarray(cosT[:, qoff : qoff + SQ]),
                "sinq": np.ascontiguousarray(sinT[:, qoff : qoff + SQ]),
                "cosk": cosT,
                "sink": sinT,
            }
        )
    return in_maps


def kernel(hidden_states, Wq, bq, Wk, bk, Wv, bv, Wo):
    from concourse.bass_utils import run_bass_kernel_spmd

    in_maps = _prepare_in_maps(hidden_states, Wq, bq, Wk, bk, Wv, bv, Wo)
    nc = _get_program()
    res = run_bass_kernel_spmd(
        nc, in_maps, core_ids=list(range(NCORES)), trace=False
    )

    out_full = np.empty((B, S, H), dtype=np.float32)
    for core in range(NCORES):
        b, tq = core // 4, core % 4
        out_full[b, tq * SQ : (tq + 1) * SQ, :] = res.results[core]["out"]
    return out_full


# revision 19
# speedup vs baseline: 2.5965x; 1.2912x over previous
"""Trainium2 Bass kernel for a GQA attention block (LuluAttention).

Problem: hidden_states [2, 2048, 2048], 16 q heads / 4 kv heads of dim 128,
RoPE, softmax attention, output projection.

Sharding: 8 cores = 2 (batch) x 4 (query-row blocks of 512 rows).
Each core computes the full K/V for its batch (all 4 kv heads), Q for its
512-row query slice (all 16 heads), RoPE, attention, and the output
projection for its row slice.  The full output is assembled on the host by
pure concatenation (no collectives needed).

Key implementation choices (vs a straightforward fp32 version):
  - All DMA'd matmul operands (hs, Wq, Wk, Wv, Wo) are bf16: 1 PE
    cycle/row (4x over fp32) and half the HBM traffic.  On-device
    intermediates (q/k/v/attn/ctx) are also bf16; PSUM accumulation stays
    fp32.  Measured end-to-end rel err ~5e-3 (gate is 2e-2).
  - hs^T is DMA'd once into SBUF and stays resident for the K and V
    projections (the fp32 version streamed it three times).
  - Everything is kept transposed ([head_dim, seq] with head_dim on SBUF
    partitions): QT/KT come straight out of matmul(lhsT=W_slice, rhs=hsT),
    scoresT = K @ Q^T, exp(scoresT) feeds the AV matmul directly
    (lhsT = V tile natural), denominator = ones @ expT, ctxT slices are
    directly the lhsT for the output projection.  No on-device transposes.
  - rotate_half for RoPE is a PE matmul with a constant +-1 permutation
    matrix R (rh = R^T @ x), replacing SBUF->SBUF half-swap DMAs.
  - Wo is prefetched into SBUF during the attention phase.
"""

import sys

if "/opt/trn_rl_repo" not in sys.path:
    sys.path.insert(0, "/opt/trn_rl_repo")

import numpy as np

B, S, H = 2, 2048, 2048
NH, NKV, D = 16, 4, 128
SQ = 512          # query rows per core
NCORES = 8
P = 128
NT = H // P       # 16 contraction tiles over hidden dim
ST = S // P       # 16 seq tiles
ROPE_THETA = 10000.0
SCALE = 1.0 / float(np.sqrt(D))
GROUPS = NH // NKV


def _np_bf16():
    from concourse import mybir

    return mybir.dt.np(mybir.dt.bfloat16)


def _rope_tables_T():
    """cosT/sinT [D, S]: transposed plain RoPE tables (the rotate-half sign
    lives in the R permutation matrix, not the tables)."""
    inv_freq = 1.0 / (ROPE_THETA ** (np.arange(0, D, 2, dtype=np.float64) / D))
    t = np.arange(S, dtype=np.float64)
    freqs = np.outer(t, inv_freq)                     # [S, D/2]
    emb = np.concatenate([freqs, freqs], axis=-1)     # [S, D]
    cos = np.cos(emb).astype(np.float32)
    sin = np.sin(emb).astype(np.float32)
    return np.ascontiguousarray(cos.T), np.ascontiguousarray(sin.T)


def _rotate_half_matrix():
    """R [128, 128] with rh = R^T @ x == rotate_half(x) for x [d, n]:
    rh[m] = -x[m+64] for m<64, rh[m] = x[m-64] for m>=64."""
    R = np.zeros((D, D), dtype=np.float32)
    for m in range(D // 2):
        R[m + D // 2, m] = -1.0
    for m in range(D // 2, D):
        R[m - D // 2, m] = 1.0
    return R


def _build_program():
    from concourse import bacc, mybir, tile

    F32 = mybir.dt.float32
    BF16 = mybir.dt.bfloat16
    AF = mybir.ActivationFunctionType

    nc = bacc.Bacc(
        "TRN2", target_bir_lowering=False, debug=False, num_devices=NCORES
    )

    # big operands come in host-tiled [128, n*cols] layouts so each loads
    # with a single large DMA: x_t[p, i*cols + c] = x[i*128 + p, c]
    hsQ = nc.dram_tensor("hsQ", [P, NT * SQ], BF16, kind="ExternalInput").ap()
    wq = nc.dram_tensor("wq", [P, NT * NH * D], BF16, kind="ExternalInput").ap()
    wk = nc.dram_tensor("wk", [P, NT * NKV * D], BF16, kind="ExternalInput").ap()
    wv = nc.dram_tensor("wv", [P, NT * NKV * D], BF16, kind="ExternalInput").ap()
    wo = nc.dram_tensor("wo", [P, NH * H], BF16, kind="ExternalInput").ap()
    bqT = nc.dram_tensor("bqT", [D, NH], F32, kind="ExternalInput").ap()
    bkT = nc.dram_tensor("bkT", [D, NKV], F32, kind="ExternalInput").ap()
    bv = nc.dram_tensor("bv", [1, NKV * D], BF16, kind="ExternalInput").ap()
    rmat = nc.dram_tensor("rmat", [D, D], BF16, kind="ExternalInput").ap()
    cosq = nc.dram_tensor("cosq", [D, SQ], F32, kind="ExternalInput").ap()
    sinq = nc.dram_tensor("sinq", [D, SQ], F32, kind="ExternalInput").ap()
    out = nc.dram_tensor("out", [SQ, H], F32, kind="ExternalOutput").ap()

    with tile.TileContext(nc) as tc:
        # ---- long-lived left-side pools ----
        cst = tc.alloc_tile_pool(name="cst", bufs=1)
        kvq = tc.alloc_tile_pool(name="kvq", bufs=1)
        ctxp = tc.alloc_tile_pool(name="ctxp", bufs=1)

        ones1 = cst.tile([1, P], BF16, tag="ones1")
        nc.gpsimd.memset(ones1[:], 1.0)
        ones128 = cst.tile([P, P], BF16, tag="ones128")
        nc.gpsimd.memset(ones128[:], 1.0)
        bqT_sb = cst.tile([D, NH], F32, tag="bqT")
        nc.scalar.dma_start(bqT_sb[:], bqT[:, :])
        bkT_sb = cst.tile([D, NKV], F32, tag="bkT")
        nc.scalar.dma_start(bkT_sb[:], bkT[:, :])
        bv_sb = cst.tile([1, NKV * D], BF16, tag="bv")
        nc.scalar.dma_start(bv_sb[:], bv[:, :])
        r_sb = cst.tile([D, D], BF16, tag="rmat")
        nc.scalar.dma_start(r_sb[:], rmat[:, :])
        cosq_sb = cst.tile([D, SQ], F32, tag="cosq")
        nc.scalar.dma_start(cosq_sb[:], cosq[:, :])
        sinq_sb = cst.tile([D, SQ], F32, tag="sinq")
        nc.scalar.dma_start(sinq_sb[:], sinq[:, :])

        # persistent bf16 intermediates
        qt = [kvq.tile([D, SQ], BF16, tag=f"qt{h}", name=f"qt{h}") for h in range(NH)]
        kt = [kvq.tile([D, S], BF16, tag=f"kt{g}", name=f"kt{g}") for g in range(NKV)]
        vt = [kvq.tile([P, NKV * D], BF16, tag=f"v{t}", name=f"v{t}") for t in range(ST)]
        ctx = [ctxp.tile([D, SQ], BF16, tag=f"ctx{h}", name=f"ctx{h}") for h in range(NH)]

        # ---- phase 1 ----
        # Each core computes K/V only for its own 512-row seq quarter
        # (which is exactly its hsQ slice), ropes it with the same tables
        # as Q, then an inter-core DRAM AllGather over the 4-core batch
        # group assembles the full K/V while the Q projection computes.
        # This removes the 4x-redundant K/V work AND the whole hsT load.
        hsp = tc.alloc_tile_pool(name="hsp", bufs=1, side="right")
        ws1 = tc.alloc_tile_pool(name="ws1", bufs=1, side="right")
        hsq = hsp.tile([P, NT * SQ], BF16, tag="hsq", name="hsq")

        # One PSUM pool set for ALL phases (3+3+2 banks): re-allocating
        # pools at a phase boundary coalesces the WAR dependency into an
        # engine-counter barrier over the whole previous phase.
        pa = tc.alloc_tile_pool(name="ps_a", bufs=3, space="PSUM")
        pc = tc.alloc_tile_pool(name="ps_c", bufs=3, space="PSUM")
        pob = tc.alloc_tile_pool(name="ps_ob", bufs=2, space="PSUM")
        rsc = tc.alloc_tile_pool(name="rsc", bufs=2, side="right")
        dcc = tc.alloc_tile_pool(name="dcc", bufs=1, space="DRAM")
        kv_in = dcc.tile([P, 8 * 512], BF16, tag="kvin", name="kv_in")
        kv_out = dcc.tile([4 * P, 8 * 512], BF16, tag="kvout", name="kv_out")

        def bank6(i, name):
            """Alternate pa/pc so up to 6 accumulators are live at once."""
            pool = pa if i % 2 == 0 else pc
            return pool.tile([P, 512], F32, tag="pj", name=name)

        def rope(dst, tbl_cos, tbl_sin):
            """dst [128, 512] bf16, in place; rh = R^T @ x on the PE."""
            rh = pob.tile([P, 512], F32, tag="po", name="rh")
            nc.tensor.matmul(rh[:], r_sb[:], dst[:], start=True, stop=True)
            t1 = rsc.tile([P, 512], F32, tag="rt1", bufs=2, name="rt1")
            nc.vector.tensor_mul(t1[:], rh[:], tbl_sin)
            t2 = rsc.tile([P, 512], F32, tag="rt2", bufs=2, name="rt2")
            nc.vector.tensor_mul(t2[:], dst[:], tbl_cos)
            nc.vector.tensor_add(dst[:], t1[:], t2[:])

        # -- K local quarter: kt_loc[g] [d, 512] = (hsq @ Wk + bk)^T --
        klo = [
            ws1.tile([D, 512], BF16, tag=f"klo{g}", name=f"klo{g}")
            for g in range(NKV)
        ]
        vlo = [
            ws1.tile([P, NKV * D], BF16, tag=f"vlo{t}", name=f"vlo{t}")
            for t in range(4)
        ]
        kbanks = {g: bank6(g, f"pk{g}") for g in range(NKV)}
        for ht in range(NT):
            nc.sync.dma_start(
                hsq[:, ht * SQ : (ht + 1) * SQ],
                hsQ[:, ht * SQ : (ht + 1) * SQ],
            )
            wkt = ws1.tile([P, NKV * D], BF16, tag="wk", bufs=3, name="wkt")
            nc.sync.dma_start(
                wkt[:], wk[:, ht * NKV * D : (ht + 1) * NKV * D]
            )
            for g in range(NKV):
                nc.tensor.matmul(
                    kbanks[g][:],
                    wkt[:, g * D : (g + 1) * D],
                    hsq[:, ht * SQ : (ht + 1) * SQ],
                    start=(ht == 0),
                    stop=(ht == NT - 1),
                )
        for g in range(NKV):
            nc.scalar.activation(
                klo[g][:], kbanks[g][:], AF.Identity,
                bias=bkT_sb[:, g : g + 1],
            )
            rope(klo[g][:], cosq_sb[:], sinq_sb[:])

        # -- V local quarter: vt_loc[ti] [128, 4*128] = hsq_sub @ Wv + bv --
        vbanks = {ti: bank6(ti, f"pv{ti}") for ti in range(4)}
        for ht in range(NT):
            wvt = ws1.tile([P, NKV * D], BF16, tag="wv", bufs=3, name="wvt")
            nc.sync.dma_start(
                wvt[:], wv[:, ht * NKV * D : (ht + 1) * NKV * D]
            )
            for ti in range(4):
                nc.tensor.matmul(
                    vbanks[ti][:],
                    hsq[:, ht * SQ + ti * P : ht * SQ + (ti + 1) * P],
                    wvt[:],
                    start=(ht == 0),
                    stop=False,
                )
        for ti in range(4):
            nc.tensor.matmul(
                vbanks[ti][:], ones1[:], bv_sb[:], start=False, stop=True
            )
            nc.scalar.copy(vlo[ti][:], vbanks[ti][:])

        # -- gather: pack -> AllGather over the batch group -> unpack.
        # All on the (otherwise idle) Pool/gpsimd queue; the Q projection
        # below overlaps the whole exchange.
        for g in range(NKV):
            nc.gpsimd.dma_start(kv_in[:, g * 512 : (g + 1) * 512], klo[g][:])
        for ti in range(4):
            nc.gpsimd.dma_start(
                kv_in[:, (4 + ti) * 512 : (5 + ti) * 512], vlo[ti][:]
            )
        nc.gpsimd.collective_compute(
            "AllGather",
            mybir.AluOpType.bypass,
            replica_groups=[[0, 1, 2, 3], [4, 5, 6, 7]],
            ins=[kv_in[:].opt()],
            outs=[kv_out[:].opt()],
        )
        for g in range(NKV):
            for k in range(4):
                nc.gpsimd.dma_start(
                    kt[g][:, k * 512 : (k + 1) * 512],
                    kv_out[k * P : (k + 1) * P, g * 512 : (g + 1) * 512],
                )
        for ti in range(4):
            for k in range(4):
                nc.gpsimd.dma_start(
                    vt[k * 4 + ti][:],
                    kv_out[k * P : (k + 1) * P, (4 + ti) * 512 : (5 + ti) * 512],
                )

        # -- Q: three sweeps of <=6 heads; wq streamed in per-sweep chunks --
        for h0, h1 in ((0, 6), (6, 12), (12, 16)):
            banks = {
                h: bank6(h - h0, f"pq{h}") for h in range(h0, h1)
            }
            for ht in range(NT):
                wqt = ws1.tile(
                    [P, 6 * P], BF16, tag="wq", bufs=3, name="wqt"
                )
                nc.sync.dma_start(
                    wqt[:, : (h1 - h0) * P],
                    wq[:, ht * NH * D + h0 * D : ht * NH * D + h1 * D],
                )
                for h in range(h0, h1):
                    nc.tensor.matmul(
                        banks[h][:],
                        wqt[:, (h - h0) * P : (h - h0 + 1) * P],
                        hsq[:, ht * SQ : (ht + 1) * SQ],
                        start=(ht == 0),
                        stop=(ht == NT - 1),
                    )
            for h in range(h0, h1):
                nc.scalar.activation(
                    qt[h][:], banks[h][:], AF.Identity,
                    bias=bqT_sb[:, h : h + 1],
                )
                rope(qt[h][:], cosq_sb[:], sinq_sb[:])
        rsc.release()
        ws1.release()
        hsp.release()

        # ---- phase 2: attention; wo prefetch streams during it ----
        wop = tc.alloc_tile_pool(name="wop", bufs=1, side="right")
        wores = wop.tile([P, NH * H], BF16, tag="wo", name="wores")
        for i in range(4):
            cs = NH * H // 4
            nc.sync.dma_start(
                wores[:, i * cs : (i + 1) * cs], wo[:, i * cs : (i + 1) * cs]
            )

        # out_acc[s1t] [128, 2048] f32 accumulates the output projection in
        # SBUF; groups of 4 heads go PSUM -> (DVE add) -> out_acc, so the
        # out-proj matmuls ride the PE's slack in the Act-bound attention
        # phase and there is no separate phase 3.
        oacc = tc.alloc_tile_pool(name="oacc", bufs=1, side="right")
        out_acc = [
            oacc.tile([P, H], F32, tag=f"oa{s}", name=f"oa{s}")
            for s in range(4)
        ]

        wsa = tc.alloc_tile_pool(name="wsa", bufs=4, side="right")
        DG = 8  # den group: sum DG exp-tiles on the DVE, 1 den matmul/group

        # Deferred out-proj emitters: each is one PE matmul (or one DVE
        # drain); they are popped one per (h, t) step so they fill the PE's
        # slack without ever blocking the Act-engine exp stream.
        pending = []

        def push_group_out(h):
            """Queue out-proj work for finished heads h-3..h."""
            for s1t in range(4):
                for hc in range(4):
                    bank_box = []

                    def mk(hh, hc=hc, s1t=s1t, h=h, bank_box=bank_box):
                        def emit():
                            if not bank_box:
                                bank_box.append(
                                    pob.tile(
                                        [P, 512], F32, tag="po", name="po"
                                    )
                                )
                            nc.tensor.matmul(
                                bank_box[0][:],
                                ctx[hh][:, s1t * P : (s1t + 1) * P],
                                wores[
                                    :,
                                    hh * H + hc * 512 : hh * H + (hc + 1) * 512,
                                ],
                                start=(hh == h - 3),
                                stop=(hh == h),
                            )

                        return emit

                    for hh in range(h - 3, h + 1):
                        emit = mk(hh)
                        emit.is_mm = True
                        pending.append(emit)

                    def drain(hc=hc, s1t=s1t, h=h, bank_box=bank_box):
                        dst = out_acc[s1t][:, hc * 512 : (hc + 1) * 512]
                        if h == 3:
                            nc.vector.tensor_copy(dst, bank_box[0][:])
                        else:
                            nc.vector.tensor_add(dst, dst, bank_box[0][:])

                    pending.append(drain)
                    if h == NH - 1:
                        def outdma(s1t=s1t, hc=hc):
                            nc.sync.dma_start(
                                out[
                                    s1t * P : (s1t + 1) * P,
                                    hc * 512 : (hc + 1) * 512,
                                ],
                                out_acc[s1t][:, hc * 512 : (hc + 1) * 512],
                            )

                        pending.append(outdma)

        def pop_pending():
            """Emit queued DVE drains freely plus one PE matmul."""
            while pending:
                fn = pending.pop(0)
                fn()
                if getattr(fn, "is_mm", False):
                    break

        for h in range(NH):
            g = h // GROUPS
            ctx_ps = pc.tile([P, SQ], F32, tag="pj", name=f"ctxps{h}")
            den_ps = pc.tile([P, SQ], F32, tag="pj", name=f"denps{h}")
            sc = [None] * ST

            def score(t):
                sc[t] = pa.tile([P, SQ], F32, tag="pj", name="sc")
                nc.tensor.matmul(
                    sc[t][:],
                    kt[g][:, t * P : (t + 1) * P],
                    qt[h][:],
                    start=True,
                    stop=True,
                )

            score(0)
            asum = None
            for t in range(ST):
                at = wsa.tile([P, SQ], BF16, tag="at", name="at")
                nc.scalar.activation(at[:], sc[t][:], AF.Exp, scale=SCALE)
                if t + 1 < ST:
                    score(t + 1)
                nc.tensor.matmul(
                    ctx_ps[:],
                    vt[t][:, g * D : (g + 1) * D],
                    at[:],
                    start=(t == 0),
                    stop=(t == ST - 1),
                )
                if t % DG == 0:
                    at0 = at
                elif t % DG == 1:
                    asum = wsa.tile([P, SQ], BF16, tag="as", bufs=2, name="asum")
                    nc.vector.tensor_add(asum[:], at0[:], at[:])
                else:
                    nc.vector.tensor_add(asum[:], asum[:], at[:])
                if t % DG == DG - 1:
                    nc.tensor.matmul(
                        den_ps[:],
                        ones128[:],
                        asum[:],
                        start=(t == DG - 1),
                        stop=(t == ST - 1),
                    )
                pop_pending()
            rc = wsa.tile([P, SQ], F32, tag="rc", bufs=2, name="rc")
            nc.vector.reciprocal(rc[:], den_ps[:])
            nc.vector.tensor_mul(ctx[h][:], ctx_ps[:], rc[:])

            if h % 4 == 3:
                push_group_out(h)
        while pending:
            pending.pop(0)()
        wsa.release()
        oacc.release()
        wop.release()
        dcc.release()
        pob.release()
        pc.release()
        pa.release()
        ctxp.release()
        kvq.release()
        cst.release()

    nc.compile()
    return nc


_PROGRAM_CACHE = {}


def _get_program():
    if "nc" not in _PROGRAM_CACHE:
        _PROGRAM_CACHE["nc"] = _build_program()
    return _PROGRAM_CACHE["nc"]


def _tile_rows(x_b):
    """[n*128, cols] -> [128, n*cols] with x_t[p, i*cols + c] = x[i*128+p, c]."""
    n = x_b.shape[0] // P
    cols = x_b.shape[1]
    return np.ascontiguousarray(
        x_b.reshape(n, P, cols).transpose(1, 0, 2).reshape(P, n * cols)
    )


def _prepare_in_maps(hidden_states, Wq, bq, Wk, bk, Wv, bv, Wo):
    bf16 = _np_bf16()
    hidden_states = np.asarray(hidden_states, dtype=np.float32)
    Wq_t = _tile_rows(np.asarray(Wq, dtype=np.float32).astype(bf16))
    Wk_t = _tile_rows(np.asarray(Wk, dtype=np.float32).astype(bf16))
    Wv_t = _tile_rows(np.asarray(Wv, dtype=np.float32).astype(bf16))
    Wo_t = _tile_rows(np.asarray(Wo, dtype=np.float32).astype(bf16))
    bq = np.asarray(bq, dtype=np.float32)
    bk = np.asarray(bk, dtype=np.float32)
    bv_b = np.asarray(bv, dtype=np.float32).astype(bf16).reshape(1, NKV * D)

    cosT, sinT = _rope_tables_T()
    rmat = _rotate_half_matrix().astype(bf16)
    bqT_h = np.ascontiguousarray(bq.reshape(NH, D).T)    # [128, 16]
    bkT_h = np.ascontiguousarray(bk.reshape(NKV, D).T)   # [128, 4]

    hsT_b = [
        np.ascontiguousarray(hidden_states[b].T).astype(bf16) for b in range(B)
    ]

    in_maps = []
    for core in range(NCORES):
        b, tq = core // 4, core % 4
        qoff = tq * SQ
        in_maps.append(
            {
                "hsQ": _tile_rows(
                    np.ascontiguousarray(hsT_b[b][:, qoff : qoff + SQ])
                ),
                "wq": Wq_t,
                "wk": Wk_t,
                "wv": Wv_t,
                "wo": Wo_t,
                "bqT": bqT_h,
                "bkT": bkT_h,
                "bv": bv_b,
                "rmat": rmat,
                "cosq": np.ascontiguousarray(cosT[:, qoff : qoff + SQ]),
                "sinq": np.ascontiguousarray(sinT[:, qoff : qoff + SQ]),
            }
        )
    return in_maps


def kernel(hidden_states, Wq, bq, Wk, bk, Wv, bv, Wo):
    from concourse.bass_utils import run_bass_kernel_spmd

    in_maps = _prepare_in_maps(hidden_states, Wq, bq, Wk, bk, Wv, bv, Wo)
    nc = _get_program()
    res = run_bass_kernel_spmd(
        nc, in_maps, core_ids=list(range(NCORES)), trace=False
    )

    out_full = np.empty((B, S, H), dtype=np.float32)
    for core in range(NCORES):
        b, tq = core // 4, core % 4
        out_full[b, tq * SQ : (tq + 1) * SQ, :] = res.results[core]["out"]
    return out_full


# revision 21
# speedup vs baseline: 2.7801x; 1.0707x over previous
"""Trainium2 Bass kernel for a GQA attention block (LuluAttention).

Problem: hidden_states [2, 2048, 2048], 16 q heads / 4 kv heads of dim 128,
RoPE, softmax attention, output projection.

Sharding: 8 cores = 2 (batch) x 4 (query-row blocks of 512 rows).
Each core computes Q for its 512-row slice (all 16 heads) and K/V for
ONLY that same 512-row seq quarter (so the whole phase reads just the
core's hsQ column slice -- no full-hsT load, no redundant K/V compute);
a DRAM AllGather over each 4-core batch group then assembles the full
roped K/V on every core, overlapped with the Q projection.  Attention
and the output projection follow per-core; the full output is a pure
host-side concatenation.

Key implementation choices (vs a straightforward fp32 version):
  - All DMA'd matmul operands (hs, Wq, Wk, Wv, Wo) are bf16: 1 PE
    cycle/row (4x over fp32) and half the HBM traffic.  On-device
    intermediates (q/k/v/attn/ctx) are also bf16; PSUM accumulation stays
    fp32.  Measured end-to-end rel err ~5e-3 (gate is 2e-2).
  - Everything is kept transposed ([head_dim, seq] with head_dim on SBUF
    partitions): QT/KT come straight out of matmul(lhsT=W_slice, rhs=hsT),
    scoresT = K @ Q^T, exp(scoresT) feeds the AV matmul directly
    (lhsT = V tile natural), denominator = ones @ expT, ctxT slices are
    directly the lhsT for the output projection.  No on-device transposes.
  - rotate_half for RoPE is a PE matmul with a constant +-1 permutation
    matrix R (rh = R^T @ x), replacing SBUF->SBUF half-swap DMAs; the
    K-quarter rows coincide with the Q rows so one table pair serves both.
  - The softmax denominator sums groups of 8 exp-tiles on the DVE first
    (bf16), leaving one ones-matmul per group on the PE.
  - The output projection is interleaved into the attention phase via a
    deferred-work queue (one matmul per attention step, accumulating in
    SBUF through a small PSUM ring), so there is no separate phase 3.
  - One PSUM pool set lives across all phases (pool re-allocation turns
    WAR deps into whole-phase engine barriers); bulk DMA rides a second
    HWDGE queue, chunked so critical transfers interleave; the collective
    and its pack/unpack run on the otherwise idle gpsimd queue.
  - Host packs all inputs into 5 tensors (each extra input buffer costs
    ~8-10us of per-exec dispatch overhead in this deployment).
"""

import sys

if "/opt/trn_rl_repo" not in sys.path:
    sys.path.insert(0, "/opt/trn_rl_repo")

import numpy as np

B, S, H = 2, 2048, 2048
NH, NKV, D = 16, 4, 128
SQ = 512          # query rows per core
NCORES = 8
P = 128
NT = H // P       # 16 contraction tiles over hidden dim
ST = S // P       # 16 seq tiles
ROPE_THETA = 10000.0
SCALE = 1.0 / float(np.sqrt(D))
GROUPS = NH // NKV


def _np_bf16():
    from concourse import mybir

    return mybir.dt.np(mybir.dt.bfloat16)


def _rope_tables_T():
    """cosT/sinT [D, S]: transposed plain RoPE tables (the rotate-half sign
    lives in the R permutation matrix, not the tables)."""
    inv_freq = 1.0 / (ROPE_THETA ** (np.arange(0, D, 2, dtype=np.float64) / D))
    t = np.arange(S, dtype=np.float64)
    freqs = np.outer(t, inv_freq)                     # [S, D/2]
    emb = np.concatenate([freqs, freqs], axis=-1)     # [S, D]
    cos = np.cos(emb).astype(np.float32)
    sin = np.sin(emb).astype(np.float32)
    return np.ascontiguousarray(cos.T), np.ascontiguousarray(sin.T)


def _rotate_half_matrix():
    """R [128, 128] with rh = R^T @ x == rotate_half(x) for x [d, n]:
    rh[m] = -x[m+64] for m<64, rh[m] = x[m-64] for m>=64."""
    R = np.zeros((D, D), dtype=np.float32)
    for m in range(D // 2):
        R[m + D // 2, m] = -1.0
    for m in range(D // 2, D):
        R[m - D // 2, m] = 1.0
    return R


def _build_program():
    from concourse import bacc, mybir, tile

    F32 = mybir.dt.float32
    BF16 = mybir.dt.bfloat16
    AF = mybir.ActivationFunctionType

    nc = bacc.Bacc(
        "TRN2", target_bir_lowering=False, debug=False, num_devices=NCORES
    )

    # big operands come in host-tiled [128, n*cols] layouts so each loads
    # with a single large DMA: x_t[p, i*cols + c] = x[i*128 + p, c]
    # inputs are consolidated into 5 tensors: each extra input buffer
    # costs ~8-10us of per-execution dispatch overhead in this runtime
    WQ_COLS = NT * NH * D
    WK_COLS = NT * NKV * D
    hsQ = nc.dram_tensor("hsQ", [P, NT * SQ], BF16, kind="ExternalInput").ap()
    wqo_t = nc.dram_tensor(
        "wqo", [P, WQ_COLS + NH * H], BF16, kind="ExternalInput"
    ).ap()
    wkv_t = nc.dram_tensor(
        "wkv", [P, 2 * WK_COLS], BF16, kind="ExternalInput"
    ).ap()
    # auxf cols: bqT[0:16] bkT[16:20] cosq[20:532] sinq[532:1044]
    auxf = nc.dram_tensor("auxf", [D, 1044], F32, kind="ExternalInput").ap()
    # auxb cols: rmat[0:128]; bv in row 0, cols [128:640]
    auxb = nc.dram_tensor("auxb", [P, 640], BF16, kind="ExternalInput").ap()
    wq = wqo_t[:, :WQ_COLS]
    wo = wqo_t[:, WQ_COLS:]
    wk = wkv_t[:, :WK_COLS]
    wv = wkv_t[:, WK_COLS:]
    bqT = auxf[:, 0:NH]
    bkT = auxf[:, NH : NH + NKV]
    cosq = auxf[:, 20:532]
    sinq = auxf[:, 532:1044]
    rmat = auxb[:, 0:D]
    bv = auxb[0:1, D : D + NKV * D]
    out = nc.dram_tensor("out", [SQ, H], F32, kind="ExternalOutput").ap()

    with tile.TileContext(nc) as tc:
        # ---- long-lived left-side pools ----
        cst = tc.alloc_tile_pool(name="cst", bufs=1)
        kvq = tc.alloc_tile_pool(name="kvq", bufs=1)
        ctxp = tc.alloc_tile_pool(name="ctxp", bufs=1)

        ones1 = cst.tile([1, P], BF16, tag="ones1")
        nc.gpsimd.memset(ones1[:], 1.0)
        ones128 = cst.tile([P, P], BF16, tag="ones128")
        nc.gpsimd.memset(ones128[:], 1.0)
        bqT_sb = cst.tile([D, NH], F32, tag="bqT")
        nc.scalar.dma_start(bqT_sb[:], bqT)
        bkT_sb = cst.tile([D, NKV], F32, tag="bkT")
        nc.scalar.dma_start(bkT_sb[:], bkT)
        bv_sb = cst.tile([1, NKV * D], BF16, tag="bv")
        nc.scalar.dma_start(bv_sb[:], bv)
        r_sb = cst.tile([D, D], BF16, tag="rmat")
        nc.scalar.dma_start(r_sb[:], rmat)
        cosq_sb = cst.tile([D, SQ], F32, tag="cosq")
        nc.scalar.dma_start(cosq_sb[:], cosq)
        sinq_sb = cst.tile([D, SQ], F32, tag="sinq")
        nc.scalar.dma_start(sinq_sb[:], sinq)

        # persistent bf16 intermediates
        qt = [kvq.tile([D, SQ], BF16, tag=f"qt{h}", name=f"qt{h}") for h in range(NH)]
        kt = [kvq.tile([D, S], BF16, tag=f"kt{g}", name=f"kt{g}") for g in range(NKV)]
        vt = [kvq.tile([P, NKV * D], BF16, tag=f"v{t}", name=f"v{t}") for t in range(ST)]
        ctx = [ctxp.tile([D, SQ], BF16, tag=f"ctx{h}", name=f"ctx{h}") for h in range(NH)]

        # ---- phase 1 ----
        # Each core computes K/V only for its own 512-row seq quarter
        # (which is exactly its hsQ slice), ropes it with the same tables
        # as Q, then an inter-core DRAM AllGather over the 4-core batch
        # group assembles the full K/V while the Q projection computes.
        # This removes the 4x-redundant K/V work AND the whole hsT load.
        hsp = tc.alloc_tile_pool(name="hsp", bufs=1, side="right")
        ws1 = tc.alloc_tile_pool(name="ws1", bufs=1, side="right")
        hsq = hsp.tile([P, NT * SQ], BF16, tag="hsq", name="hsq")

        # One PSUM pool set for ALL phases (3+3+2 banks): re-allocating
        # pools at a phase boundary coalesces the WAR dependency into an
        # engine-counter barrier over the whole previous phase.
        pa = tc.alloc_tile_pool(name="ps_a", bufs=3, space="PSUM")
        pc = tc.alloc_tile_pool(name="ps_c", bufs=3, space="PSUM")
        pob = tc.alloc_tile_pool(name="ps_ob", bufs=2, space="PSUM")
        rsc = tc.alloc_tile_pool(name="rsc", bufs=2, side="right")
        dcc = tc.alloc_tile_pool(name="dcc", bufs=1, space="DRAM")
        kv_in = dcc.tile([P, 8 * 512], BF16, tag="kvin", name="kv_in")
        kv_out = dcc.tile([4 * P, 8 * 512], BF16, tag="kvout", name="kv_out")

        def bank6(i, name):
            """Alternate pa/pc so up to 6 accumulators are live at once."""
            pool = pa if i % 2 == 0 else pc
            return pool.tile([P, 512], F32, tag="pj", name=name)

        def rope(dst, tbl_cos, tbl_sin):
            """dst [128, 512] bf16, in place; rh = R^T @ x on the PE."""
            rh = pob.tile([P, 512], F32, tag="po", name="rh")
            nc.tensor.matmul(rh[:], r_sb[:], dst[:], start=True, stop=True)
            t1 = rsc.tile([P, 512], F32, tag="rt1", bufs=2, name="rt1")
            nc.vector.tensor_mul(t1[:], rh[:], tbl_sin)
            t2 = rsc.tile([P, 512], F32, tag="rt2", bufs=2, name="rt2")
            nc.vector.tensor_mul(t2[:], dst[:], tbl_cos)
            nc.vector.tensor_add(dst[:], t1[:], t2[:])

        # -- K local quarter: kt_loc[g] [d, 512] = (hsq @ Wk + bk)^T --
        klo = [
            ws1.tile([D, 512], BF16, tag=f"klo{g}", name=f"klo{g}")
            for g in range(NKV)
        ]
        vlo = [
            ws1.tile([P, NKV * D], BF16, tag=f"vlo{t}", name=f"vlo{t}")
            for t in range(4)
        ]
        kbanks = {g: bank6(g, f"pk{g}") for g in range(NKV)}
        for ht in range(NT):
            nc.sync.dma_start(
                hsq[:, ht * SQ : (ht + 1) * SQ],
                hsQ[:, ht * SQ : (ht + 1) * SQ],
            )
            wkt = ws1.tile([P, NKV * D], BF16, tag="wk", bufs=3, name="wkt")
            nc.sync.dma_start(
                wkt[:], wk[:, ht * NKV * D : (ht + 1) * NKV * D]
            )
            for g in range(NKV):
                nc.tensor.matmul(
                    kbanks[g][:],
                    wkt[:, g * D : (g + 1) * D],
                    hsq[:, ht * SQ : (ht + 1) * SQ],
                    start=(ht == 0),
                    stop=(ht == NT - 1),
                )
        for g in range(NKV):
            nc.scalar.activation(
                klo[g][:], kbanks[g][:], AF.Identity,
                bias=bkT_sb[:, g : g + 1],
            )
            rope(klo[g][:], cosq_sb[:], sinq_sb[:])

        # -- V local quarter: vt_loc[ti] [128, 4*128] = hsq_sub @ Wv + bv --
        vbanks = {ti: bank6(ti, f"pv{ti}") for ti in range(4)}
        for ht in range(NT):
            wvt = ws1.tile([P, NKV * D], BF16, tag="wv", bufs=3, name="wvt")
            nc.sync.dma_start(
                wvt[:], wv[:, ht * NKV * D : (ht + 1) * NKV * D]
            )
            for ti in range(4):
                nc.tensor.matmul(
                    vbanks[ti][:],
                    hsq[:, ht * SQ + ti * P : ht * SQ + (ti + 1) * P],
                    wvt[:],
                    start=(ht == 0),
                    stop=False,
                )
        for ti in range(4):
            nc.tensor.matmul(
                vbanks[ti][:], ones1[:], bv_sb[:], start=False, stop=True
            )
            nc.scalar.copy(vlo[ti][:], vbanks[ti][:])

        # -- gather: pack -> AllGather over the batch group -> unpack.
        # All on the (otherwise idle) Pool/gpsimd queue; the Q projection
        # below overlaps the whole exchange.
        for g in range(NKV):
            nc.gpsimd.dma_start(kv_in[:, g * 512 : (g + 1) * 512], klo[g][:])
        for ti in range(4):
            nc.gpsimd.dma_start(
                kv_in[:, (4 + ti) * 512 : (5 + ti) * 512], vlo[ti][:]
            )
        nc.gpsimd.collective_compute(
            "AllGather",
            mybir.AluOpType.bypass,
            replica_groups=[[0, 1, 2, 3], [4, 5, 6, 7]],
            ins=[kv_in[:].opt()],
            outs=[kv_out[:].opt()],
        )
        for g in range(NKV):
            for k in range(4):
                nc.gpsimd.dma_start(
                    kt[g][:, k * 512 : (k + 1) * 512],
                    kv_out[k * P : (k + 1) * P, g * 512 : (g + 1) * 512],
                )
        for ti in range(4):
            for k in range(4):
                nc.gpsimd.dma_start(
                    vt[k * 4 + ti][:],
                    kv_out[k * P : (k + 1) * P, (4 + ti) * 512 : (5 + ti) * 512],
                )

        # -- Q: three sweeps of <=6 heads; wq streamed in per-sweep chunks --
        for h0, h1 in ((0, 6), (6, 12), (12, 16)):
            banks = {
                h: bank6(h - h0, f"pq{h}") for h in range(h0, h1)
            }
            for ht in range(NT):
                wqt = ws1.tile(
                    [P, 6 * P], BF16, tag="wq", bufs=3, name="wqt"
                )
                nc.sync.dma_start(
                    wqt[:, : (h1 - h0) * P],
                    wq[:, ht * NH * D + h0 * D : ht * NH * D + h1 * D],
                )
                for h in range(h0, h1):
                    nc.tensor.matmul(
                        banks[h][:],
                        wqt[:, (h - h0) * P : (h - h0 + 1) * P],
                        hsq[:, ht * SQ : (ht + 1) * SQ],
                        start=(ht == 0),
                        stop=(ht == NT - 1),
                    )
            for h in range(h0, h1):
                nc.scalar.activation(
                    qt[h][:], banks[h][:], AF.Identity,
                    bias=bqT_sb[:, h : h + 1],
                )
                rope(qt[h][:], cosq_sb[:], sinq_sb[:])
        rsc.release()
        ws1.release()
        hsp.release()

        # ---- phase 2: attention; wo prefetch streams during it ----
        wop = tc.alloc_tile_pool(name="wop", bufs=1, side="right")
        wores = wop.tile([P, NH * H], BF16, tag="wo", name="wores")
        for i in range(4):
            cs = NH * H // 4
            nc.sync.dma_start(
                wores[:, i * cs : (i + 1) * cs], wo[:, i * cs : (i + 1) * cs]
            )

        # out_acc[s1t] [128, 2048] f32 accumulates the output projection in
        # SBUF; groups of 4 heads go PSUM -> (DVE add) -> out_acc, so the
        # out-proj matmuls ride the PE's slack in the Act-bound attention
        # phase and there is no separate phase 3.
        oacc = tc.alloc_tile_pool(name="oacc", bufs=1, side="right")
        out_acc = [
            oacc.tile([P, H], F32, tag=f"oa{s}", name=f"oa{s}")
            for s in range(4)
        ]

        wsa = tc.alloc_tile_pool(name="wsa", bufs=4, side="right")
        DG = 8  # den group: sum DG exp-tiles on the DVE, 1 den matmul/group

        # Deferred out-proj emitters: each is one PE matmul (or one DVE
        # drain); they are popped one per (h, t) step so they fill the PE's
        # slack without ever blocking the Act-engine exp stream.
        pending = []

        def push_group_out(h):
            """Queue out-proj work for finished heads h-3..h."""
            for s1t in range(4):
                for hc in range(4):
                    bank_box = []

                    def mk(hh, hc=hc, s1t=s1t, h=h, bank_box=bank_box):
                        def emit():
                            if not bank_box:
                                bank_box.append(
                                    pob.tile(
                                        [P, 512], F32, tag="po", name="po"
                                    )
                                )
                            nc.tensor.matmul(
                                bank_box[0][:],
                                ctx[hh][:, s1t * P : (s1t + 1) * P],
                                wores[
                                    :,
                                    hh * H + hc * 512 : hh * H + (hc + 1) * 512,
                                ],
                                start=(hh == h - 3),
                                stop=(hh == h),
                            )

                        return emit

                    for hh in range(h - 3, h + 1):
                        emit = mk(hh)
                        emit.is_mm = True
                        pending.append(emit)

                    def drain(hc=hc, s1t=s1t, h=h, bank_box=bank_box):
                        dst = out_acc[s1t][:, hc * 512 : (hc + 1) * 512]
                        if h == 3:
                            nc.vector.tensor_copy(dst, bank_box[0][:])
                        else:
                            nc.vector.tensor_add(dst, dst, bank_box[0][:])

                    pending.append(drain)
                    if h == NH - 1:
                        def outdma(s1t=s1t, hc=hc):
                            nc.sync.dma_start(
                                out[
                                    s1t * P : (s1t + 1) * P,
                                    hc * 512 : (hc + 1) * 512,
                                ],
                                out_acc[s1t][:, hc * 512 : (hc + 1) * 512],
                            )

                        pending.append(outdma)

        def pop_pending():
            """Emit queued DVE drains freely plus one PE matmul."""
            while pending:
                fn = pending.pop(0)
                fn()
                if getattr(fn, "is_mm", False):
                    break

        for h in range(NH):
            g = h // GROUPS
            ctx_ps = pc.tile([P, SQ], F32, tag="pj", name=f"ctxps{h}")
            den_ps = pc.tile([P, SQ], F32, tag="pj", name=f"denps{h}")
            sc = [None] * ST

            def score(t):
                sc[t] = pa.tile([P, SQ], F32, tag="pj", name="sc")
                nc.tensor.matmul(
                    sc[t][:],
                    kt[g][:, t * P : (t + 1) * P],
                    qt[h][:],
                    start=True,
                    stop=True,
                )

            score(0)
            asum = None
            for t in range(ST):
                at = wsa.tile([P, SQ], BF16, tag="at", name="at")
                nc.scalar.activation(at[:], sc[t][:], AF.Exp, scale=SCALE)
                if t + 1 < ST:
                    score(t + 1)
                nc.tensor.matmul(
                    ctx_ps[:],
                    vt[t][:, g * D : (g + 1) * D],
                    at[:],
                    start=(t == 0),
                    stop=(t == ST - 1),
                )
                if t % DG == 0:
                    at0 = at
                elif t % DG == 1:
                    asum = wsa.tile([P, SQ], BF16, tag="as", bufs=2, name="asum")
                    nc.vector.tensor_add(asum[:], at0[:], at[:])
                else:
                    nc.vector.tensor_add(asum[:], asum[:], at[:])
                if t % DG == DG - 1:
                    nc.tensor.matmul(
                        den_ps[:],
                        ones128[:],
                        asum[:],
                        start=(t == DG - 1),
                        stop=(t == ST - 1),
                    )
                pop_pending()
            rc = wsa.tile([P, SQ], F32, tag="rc", bufs=2, name="rc")
            nc.vector.reciprocal(rc[:], den_ps[:])
            nc.vector.tensor_mul(ctx[h][:], ctx_ps[:], rc[:])

            if h % 4 == 3:
                push_group_out(h)
        while pending:
            pending.pop(0)()
        wsa.release()
        oacc.release()
        wop.release()
        dcc.release()
        pob.release()
        pc.release()
        pa.release()
        ctxp.release()
        kvq.release()
        cst.release()

    nc.compile()
    return nc


_PROGRAM_CACHE = {}


def _get_program():
    if "nc" not in _PROGRAM_CACHE:
        _PROGRAM_CACHE["nc"] = _build_program()
    return _PROGRAM_CACHE["nc"]


def _tile_rows(x_b):
    """[n*128, cols] -> [128, n*cols] with x_t[p, i*cols + c] = x[i*128+p, c]."""
    n = x_b.shape[0] // P
    cols = x_b.shape[1]
    return np.ascontiguousarray(
        x_b.reshape(n, P, cols).transpose(1, 0, 2).reshape(P, n * cols)
    )


def _prepare_in_maps(hidden_states, Wq, bq, Wk, bk, Wv, bv, Wo):
    bf16 = _np_bf16()
    hidden_states = np.asarray(hidden_states, dtype=np.float32)
    Wq_t = _tile_rows(np.asarray(Wq, dtype=np.float32).astype(bf16))
    Wk_t = _tile_rows(np.asarray(Wk, dtype=np.float32).astype(bf16))
    Wv_t = _tile_rows(np.asarray(Wv, dtype=np.float32).astype(bf16))
    Wo_t = _tile_rows(np.asarray(Wo, dtype=np.float32).astype(bf16))
    wqo_h = np.concatenate([Wq_t, Wo_t], axis=1)
    wkv_h = np.concatenate([Wk_t, Wv_t], axis=1)
    bq = np.asarray(bq, dtype=np.float32)
    bk = np.asarray(bk, dtype=np.float32)
    bv_b = np.asarray(bv, dtype=np.float32).astype(bf16).reshape(1, NKV * D)

    cosT, sinT = _rope_tables_T()
    rmat = _rotate_half_matrix().astype(bf16)
    bqT_h = np.ascontiguousarray(bq.reshape(NH, D).T)    # [128, 16]
    bkT_h = np.ascontiguousarray(bk.reshape(NKV, D).T)   # [128, 4]
    auxb_h = np.zeros((P, 640), bf16)
    auxb_h[:, :D] = rmat
    auxb_h[0:1, D : D + NKV * D] = bv_b

    hsT_b = [
        np.ascontiguousarray(hidden_states[b].T).astype(bf16) for b in range(B)
    ]

    in_maps = []
    for core in range(NCORES):
        b, tq = core // 4, core % 4
        qoff = tq * SQ
        auxf_h = np.zeros((D, 1044), np.float32)
        auxf_h[:, :NH] = bqT_h
        auxf_h[:, NH : NH + NKV] = bkT_h
        auxf_h[:, 20:532] = cosT[:, qoff : qoff + SQ]
        auxf_h[:, 532:1044] = sinT[:, qoff : qoff + SQ]
        in_maps.append(
            {
                "hsQ": _tile_rows(
                    np.ascontiguousarray(hsT_b[b][:, qoff : qoff + SQ])
                ),
                "wqo": wqo_h,
                "wkv": wkv_h,
                "auxf": auxf_h,
                "auxb": auxb_h,
            }
        )
    return in_maps


def kernel(hidden_states, Wq, bq, Wk, bk, Wv, bv, Wo):
    from concourse.bass_utils import run_bass_kernel_spmd

    in_maps = _prepare_in_maps(hidden_states, Wq, bq, Wk, bk, Wv, bv, Wo)
    nc = _get_program()
    res = run_bass_kernel_spmd(
        nc, in_maps, core_ids=list(range(NCORES)), trace=False
    )

    out_full = np.empty((B, S, H), dtype=np.float32)
    for core in range(NCORES):
        b, tq = core // 4, core % 4
        out_full[b, tq * SQ : (tq + 1) * SQ, :] = res.results[core]["out"]
    return out_full
